# revision 12
# baseline (speedup 1.0000x reference)
"""Trainium2 Bass kernel for MetaLayer-style GNN (edge/node/global GRU message passing).

Contract: kernel(**inputs) takes the FULL unsharded inputs (np arrays, keys as in
setup_inputs) and returns the FULL output [B, STEPS, H] float32.

Strategy (8 NeuronCores):
- Sort edges by dst, shard nodes into 8 equal contiguous ranges; each core owns all
  edges whose dst is in its range => node aggregation is core-local.
- Per step: edge MLP+GRU (edge-parallel, bf16 matmuls, T-form activations),
  local segment-sum via PE-transpose + dma_scatter_add (fp32, DMA CCE adds),
  node MLP+GRU on local nodes, AllGather of updated x (bf16) to rebuild the
  replicated gather tables, small AllReduce for per-graph node means, replicated
  global MLP+GRU on every core.
- x and u kept resident in fp32 SBUF; MLP second layer folded into GRU input
  weights: gi = relu_h1 @ (Wih@W2).T + (Wih@b2 + bih).
"""

from contextlib import ExitStack

import numpy as np
import ml_dtypes

import concourse.bass as bass
import concourse.bacc as bacc
import concourse.tile as tile
from concourse import mybir
from concourse.bass_utils import run_bass_kernel_spmd
from concourse.masks import make_identity

BF16 = ml_dtypes.bfloat16
AF = mybir.ActivationFunctionType
DT = mybir.dt
ALU = mybir.AluOpType

# ---------------------------------------------------------------- configuration

class Cfg:
    def __init__(self, N=50000, E=500000, B=64, H=128, STEPS=3, NCORES=8,
                 CH=512, SCB=4096):
        assert H == 128
        assert N % NCORES == 0
        self.N, self.E, self.B, self.H, self.STEPS, self.NCORES = N, E, B, H, STEPS, NCORES
        self.CH = CH                      # edge chunk (free dim of f32 PSUM <= 512)
        self.SCB = SCB                    # edges per dma_scatter_add call
        self.NL = N // NCORES             # local nodes
        self.NLP = ((self.NL + CH - 1) // CH) * CH
        self.NCHN = self.NLP // CH        # node chunks
        self.LO_REAL = min(N, 32767)      # x rows in lo table (int16 index limit)
        self.HI_REAL = N - self.LO_REAL
        self.LO_ROWS = self.LO_REAL + 1   # + zero row
        self.HI_ROWS = self.HI_REAL + 1   # + zero row
        self.TLOC_ROWS = self.NL + 1      # + zero row

    def finalize(self, max_shard_edges):
        assert self.SCB % self.CH == 0
        self.EPAD = ((max_shard_edges + self.SCB - 1) // self.SCB) * self.SCB
        self.NCHE = self.EPAD // self.CH  # edge chunks
        self.NSCB = self.EPAD // self.SCB # scatter calls per step
        self.CPB = self.SCB // self.CH    # chunks per scatter block
        return self


# ---------------------------------------------------------------- host helpers

def _wrap16(idx, call):
    """Pack indices into the wrapped-16, replicated-128 layout of dma_gather /
    dma_scatter_add: element [p, c*(call//16) + s] = idx[c*call + s*16 + p%16]."""
    total = idx.shape[0]
    assert total % call == 0 and call % 16 == 0
    ncalls = total // call
    w = idx.reshape(ncalls, call // 16, 16)                   # [c, s, lane]
    w = np.transpose(w, (2, 0, 1)).reshape(16, total // 16)   # [lane, c*s]
    w = np.tile(w, (8, 1))                                    # -> 128 partitions
    return np.ascontiguousarray(w.astype(np.int16))


def _onehot(cols_idx, nrows, scale=None, dtype=BF16):
    """[nrows, len(cols_idx)]: out[cols_idx[j], j] = scale_j; idx<0 -> zero col."""
    ncols = cols_idx.shape[0]
    out = np.zeros((nrows, ncols), dtype=np.float32)
    j = np.nonzero(cols_idx >= 0)[0]
    s = np.ones(j.shape[0], np.float32) if scale is None else scale[j]
    out[cols_idx[j], j] = s
    return out.astype(dtype)


def host_prepare(cfg, inputs):
    N, E, B, H = cfg.N, cfg.E, cfg.B, cfg.H
    x = np.asarray(inputs['x'], np.float32)
    edge_index = np.asarray(inputs['edge_index'])
    edge_attr = np.asarray(inputs['edge_attr'], np.float32)
    u = np.asarray(inputs['u'], np.float32)
    batch = np.asarray(inputs['batch']).astype(np.int64)
    src, dst = edge_index[0].astype(np.int64), edge_index[1].astype(np.int64)

    def g(name):
        return np.asarray(inputs[name], np.float32)

    W1, b1 = g('edge_w1'), g('edge_b1')
    W2, b2 = g('edge_w2'), g('edge_b2')
    eWih, eWhh = g('egru_wih'), g('egru_whh')
    eBih, eBhh = g('egru_bih'), g('egru_bhh')
    nW1, nb1 = g('node_w1'), g('node_b1')
    nW2, nb2 = g('node_w2'), g('node_b2')
    nWih, nWhh = g('ngru_wih'), g('ngru_whh')
    nBih, nBhh = g('ngru_bih'), g('ngru_bhh')
    gW1, gb1 = g('glob_w1'), g('glob_b1')
    gW2, gb2 = g('glob_w2'), g('glob_b2')
    gWih, gWhh = g('ggru_wih'), g('ggru_whh')
    gBih, gBhh = g('ggru_bih'), g('ggru_bhh')

    eWih2, eBih2 = eWih @ W2, eWih @ b2 + eBih
    nWih2, nBih2 = nWih @ nW2, nWih @ nb2 + nBih
    gWih2, gBih2 = gWih @ gW2, gWih @ gb2 + gBih

    def gate(Wm, i):
        return Wm[i * H:(i + 1) * H, :].T

    blocks = [
        W1[:, 0:H].T, W1[:, H:2 * H].T, W1[:, 2 * H:3 * H].T, W1[:, 3 * H:4 * H].T,
        gate(eWih2, 0), gate(eWih2, 1), gate(eWih2, 2),
        gate(eWhh, 0), gate(eWhh, 1), gate(eWhh, 2),
        nW1[:, 0:H].T, nW1[:, H:2 * H].T, nW1[:, 2 * H:3 * H].T,
        gate(nWih2, 0), gate(nWih2, 1), gate(nWih2, 2),
        gate(nWhh, 0), gate(nWhh, 1), gate(nWhh, 2),
        gW1[:, 0:H].T, gW1[:, H:2 * H].T,
        gate(gWih2, 0), gate(gWih2, 1), gate(gWih2, 2),
        gate(gWhh, 0), gate(gWhh, 1), gate(gWhh, 2),
    ]
    wpk = np.concatenate([bl.astype(np.float32) for bl in blocks], axis=1).astype(BF16)

    def gb_(v, i):
        return v[i * H:(i + 1) * H]

    bcols = [
        b1, gb_(eBih2, 0) + gb_(eBhh, 0), gb_(eBih2, 1) + gb_(eBhh, 1), gb_(eBhh, 2), gb_(eBih2, 2),
        nb1, gb_(nBih2, 0) + gb_(nBhh, 0), gb_(nBih2, 1) + gb_(nBhh, 1), gb_(nBhh, 2), gb_(nBih2, 2),
        gb1, gb_(gBih2, 0) + gb_(gBhh, 0), gb_(gBih2, 1) + gb_(gBhh, 1), gb_(gBhh, 2), gb_(gBih2, 2),
    ]
    bpk = np.stack(bcols, axis=1).astype(np.float32)

    order = np.argsort(dst, kind='stable')
    ssrc, sdst, sea = src[order], dst[order], edge_attr[order]
    shard_of = sdst // cfg.NL
    counts = np.bincount(shard_of, minlength=cfg.NCORES)
    cfg.finalize(int(counts.max()))

    gcnt = np.bincount(batch, minlength=B).astype(np.float32)
    ginv = 1.0 / np.maximum(gcnt, 1.0)
    ncnt = np.bincount(sdst, minlength=N).astype(np.float32)
    ninv = 1.0 / np.maximum(ncnt, 1.0)
    bsrc_all = batch[ssrc]

    xb = x.astype(BF16)
    in_maps = []
    bounds = np.searchsorted(sdst, np.arange(cfg.NCORES + 1) * cfg.NL)
    for c in range(cfg.NCORES):
        lo_, hi_ = int(bounds[c]), int(bounds[c + 1])
        ne = hi_ - lo_
        npad = cfg.EPAD - ne
        base = c * cfg.NL

        csrc = ssrc[lo_:hi_]
        cdst_loc = sdst[lo_:hi_] - base
        cbsrc = bsrc_all[lo_:hi_]

        src_lo = np.where(csrc < cfg.LO_REAL, csrc, cfg.LO_REAL)
        src_hi = np.where(csrc >= cfg.LO_REAL, csrc - cfg.LO_REAL, cfg.HI_REAL)
        src_lo = np.concatenate([src_lo, np.full(npad, cfg.LO_REAL, np.int64)])
        src_hi = np.concatenate([src_hi, np.full(npad, cfg.HI_REAL, np.int64)])
        dst_loc = np.concatenate([cdst_loc, np.full(npad, cfg.NL, np.int64)])
        scat = np.concatenate([cdst_loc, np.full(npad, min(cfg.NL, cfg.NLP - 1), np.int64)])

        nl, nlp = cfg.NL, cfg.NLP
        batch_loc = batch[base:base + nl]
        bl_pad = np.concatenate([batch_loc, np.full(nlp - nl, -1, np.int64)])

        invc = np.zeros(nlp, np.float32)
        invc[:nl] = ninv[base:base + nl]

        xT0 = np.zeros((128, nlp), np.float32)
        xT0[:, :nl] = x[base:base + nl].T
        eT0 = np.zeros((128, cfg.EPAD), BF16)
        eT0[:, :ne] = sea[lo_:hi_].T.astype(BF16)

        in_maps.append(dict(
            wpk=wpk, bpk=bpk,
            xT0=xT0,
            uT0=np.ascontiguousarray(u.T).astype(np.float32),
            eT0=eT0,
            Tlo0=np.concatenate([xb[:cfg.LO_REAL], np.zeros((1, H), BF16)], axis=0),
            Thi0=np.concatenate([xb[cfg.LO_REAL:], np.zeros((1, H), BF16)], axis=0),
            Tloc0=np.concatenate([xb[base:base + nl], np.zeros((1, H), BF16)], axis=0),
            gsrc_lo=_wrap16(src_lo, cfg.CH),
            gsrc_hi=_wrap16(src_hi, cfg.CH),
            gdst=_wrap16(dst_loc, cfg.CH),
            sscat=_wrap16(scat, cfg.SCB),
            S_u=_onehot(np.concatenate([cbsrc, np.full(npad, -1, np.int64)]), B),
            S_nb=_onehot(bl_pad, B),
            Bmat=np.ascontiguousarray(
                _onehot(bl_pad, B, scale=ginv[np.clip(bl_pad, 0, B - 1)]).T),
            invcT=np.ascontiguousarray(
                np.broadcast_to(invc[None, :], (128, nlp))).astype(BF16),
            agg_zero=np.zeros((cfg.NLP, H), np.float32),
        ))
    return in_maps


# ---------------------------------------------------------------- device program

def build_program(cfg):
    nc = bacc.Bacc("TRN2", target_bir_lowering=False, debug=False,
                   num_devices=cfg.NCORES, num_swdge_queues=4)
    H, B, CH = cfg.H, cfg.B, cfg.CH
    NW = 27
    f32, bf16, i16 = DT.float32, DT.bfloat16, DT.int16

    def din(name, shape, dt):
        return nc.dram_tensor(name, shape, dt, kind="ExternalInput").ap()

    t = {}
    t['wpk'] = din("wpk", [128, NW * 128], bf16)
    t['bpk'] = din("bpk", [128, 15], f32)
    t['xT0'] = din("xT0", [128, cfg.NLP], f32)
    t['uT0'] = din("uT0", [128, B], f32)
    t['eT0'] = din("eT0", [128, cfg.EPAD], bf16)
    t['Tlo0'] = din("Tlo0", [cfg.LO_ROWS, H], bf16)
    t['Thi0'] = din("Thi0", [cfg.HI_ROWS, H], bf16)
    t['Tloc0'] = din("Tloc0", [cfg.TLOC_ROWS, H], bf16)
    t['gsrc_lo'] = din("gsrc_lo", [128, cfg.EPAD // 16], i16)
    t['gsrc_hi'] = din("gsrc_hi", [128, cfg.EPAD // 16], i16)
    t['gdst'] = din("gdst", [128, cfg.EPAD // 16], i16)
    t['sscat'] = din("sscat", [128, cfg.EPAD // 16], i16)
    t['S_u'] = din("S_u", [B, cfg.EPAD], bf16)
    t['S_nb'] = din("S_nb", [B, cfg.NLP], bf16)
    t['Bmat'] = din("Bmat", [cfg.NLP, B], bf16)
    t['invcT'] = din("invcT", [128, cfg.NLP], bf16)
    t['agg_zero'] = din("agg_zero", [cfg.NLP, H], f32)

    t['out'] = nc.dram_tensor("out", [B, cfg.STEPS, H], f32, kind="ExternalOutput").ap()

    t['eTd'] = [nc.dram_tensor(f"eTd{i}", [128, cfg.EPAD], bf16).ap() for i in range(2)]
    t['Tlo'] = nc.dram_tensor("Tlo", [cfg.LO_ROWS, H], bf16).ap()
    t['Thi'] = nc.dram_tensor("Thi", [cfg.HI_ROWS, H], bf16).ap()
    t['Tloc'] = nc.dram_tensor("Tloc", [cfg.TLOC_ROWS, H], bf16).ap()
    t['aggd'] = [nc.dram_tensor(f"aggd{i}", [cfg.NLP, H], f32).ap() for i in range(2)]
    t['x_shard'] = nc.dram_tensor("x_shard", [cfg.NL, H], bf16).ap()
    t['x_full'] = nc.dram_tensor("x_full", [cfg.N, H], bf16, addr_space="Shared").ap()
    t['gsum_in'] = nc.dram_tensor("gsum_in", [128, B], f32).ap()
    t['gsum_out'] = nc.dram_tensor("gsum_out", [128, B], f32, addr_space="Shared").ap()
    t['rg'] = [list(range(cfg.NCORES))]

    with ExitStack() as ctx:
        tc = ctx.enter_context(tile.TileContext(nc))
        _emit(nc, tc, ctx, cfg, t)
    nc.compile()
    return nc


def _emit(nc, tc, ctx, cfg, t):
    H, B, CH = cfg.H, cfg.B, cfg.CH
    f32, bf16, i16 = DT.float32, DT.bfloat16, DT.int16
    NSUB = CH // 128

    perm = ctx.enter_context(tc.tile_pool(name="perm", bufs=1))
    sb = ctx.enter_context(tc.tile_pool(name="sb", bufs=2))
    sb2 = ctx.enter_context(tc.tile_pool(name="sb2", bufs=2))
    stg = ctx.enter_context(tc.tile_pool(name="stg", bufs=2))
    ps_h1 = ctx.enter_context(tc.tile_pool(name="ps_h1", bufs=2, space="PSUM"))
    ps_g = ctx.enter_context(tc.tile_pool(name="ps_g", bufs=1, space="PSUM"))
    ps_tp = ctx.enter_context(tc.tile_pool(name="ps_tp", bufs=1, space="PSUM"))

    # ---------------- persistent SBUF state
    W = perm.tile([128, 27 * 128], bf16)
    nc.sync.dma_start(W[:], t['wpk'][:])

    def w(i):
        return W[:, i * 128:(i + 1) * 128]

    bias = perm.tile([128, 15], f32)
    nc.sync.dma_start(bias[:], t['bpk'][:])

    def bv(i):
        return bias[:, i:i + 1]

    xT = perm.tile([128, cfg.NLP], f32)
    nc.sync.dma_start(xT[:], t['xT0'][:])
    xTb = perm.tile([128, cfg.NLP], bf16)
    nc.vector.tensor_copy(xTb[:], xT[:])

    uT = perm.tile([128, B], f32)
    nc.sync.dma_start(uT[:], t['uT0'][:])
    uTb = perm.tile([128, B], bf16)
    nc.vector.tensor_copy(uTb[:], uT[:])

    bsum_acc = perm.tile([128, B], f32)

    ident_f = perm.tile([128, 128], f32)
    make_identity(nc, ident_f[:])
    ident_b = perm.tile([128, 128], bf16)
    nc.vector.tensor_copy(ident_b[:], ident_f[:])

    # ---------------- init DRAM state
    nc.sync.dma_start(t['eTd'][0][:], t['eT0'][:])
    nc.sync.dma_start(t['Tlo'][:], t['Tlo0'][:])
    nc.sync.dma_start(t['Thi'][:], t['Thi0'][:])
    nc.sync.dma_start(t['Tloc'][:], t['Tloc0'][:])

    def gru(xiT, hTb, wb, bb, pool, h_f32, out_tag, FD):
        """GRU tail: xiT bf16 [128,FD] (input through W2 fold), hTb bf16 [128,FD].
        If h_f32 given: blend in f32 in-place there and return None; else return
        a bf16 tile. wb: base index of Wih2 r,z,g then Whh r,z,g. bb: bias base."""
        pr = ps_g.tile([128, FD], f32, tag="pr")
        nc.tensor.matmul(pr[:], lhsT=w(wb + 0), rhs=xiT, start=True, stop=False)
        nc.tensor.matmul(pr[:], lhsT=w(wb + 3), rhs=hTb, start=False, stop=True)
        pz = ps_g.tile([128, FD], f32, tag="pz")
        nc.tensor.matmul(pz[:], lhsT=w(wb + 1), rhs=xiT, start=True, stop=False)
        nc.tensor.matmul(pz[:], lhsT=w(wb + 4), rhs=hTb, start=False, stop=True)
        pig = ps_g.tile([128, FD], f32, tag="pig")
        nc.tensor.matmul(pig[:], lhsT=w(wb + 2), rhs=xiT, start=True, stop=True)
        phg = ps_g.tile([128, FD], f32, tag="phg")
        nc.tensor.matmul(phg[:], lhsT=w(wb + 5), rhs=hTb, start=True, stop=True)

        r = pool.tile([128, FD], bf16, tag="r")
        nc.scalar.activation(r[:], pr[:], AF.Sigmoid, bias=bv(bb + 0))
        z = pool.tile([128, FD], bf16, tag="z")
        nc.scalar.activation(z[:], pz[:], AF.Sigmoid, bias=bv(bb + 1))
        hg = pool.tile([128, FD], bf16, tag="hg")
        nc.scalar.activation(hg[:], phg[:], AF.Identity, bias=bv(bb + 2))
        tm = pool.tile([128, FD], f32, tag="tm")
        nc.vector.tensor_tensor(tm[:], r[:], hg[:], op=ALU.mult)
        sp = pool.tile([128, FD], f32, tag="sp")
        nc.vector.tensor_tensor(sp[:], tm[:], pig[:], op=ALU.add)
        n = pool.tile([128, FD], bf16, tag="n")
        nc.scalar.activation(n[:], sp[:], AF.Tanh, bias=bv(bb + 3))

        hold = h_f32 if h_f32 is not None else hTb
        d = pool.tile([128, FD], f32, tag="d")
        nc.vector.tensor_tensor(d[:], hold, n[:], op=ALU.subtract)
        m = pool.tile([128, FD], f32, tag="m")
        nc.vector.tensor_tensor(m[:], z[:], d[:], op=ALU.mult)
        if h_f32 is not None:
            nc.vector.tensor_tensor(h_f32, n[:], m[:], op=ALU.add)
            return None
        hN = pool.tile([128, FD], bf16, tag=out_tag)
        nc.vector.tensor_tensor(hN[:], n[:], m[:], op=ALU.add)
        return hN

    for s in range(cfg.STEPS):
        eT_r, eT_w = t['eTd'][s % 2], t['eTd'][(s + 1) % 2]
        agg = t['aggd'][s % 2]
        nc.sync.dma_start(agg[:], t['agg_zero'][:])

        # per-step u projections: uWd_row = u @ W1d.T ; uWnc_row = u @ Wn1c.T
        uprj = []
        for wi, tg in ((3, "uprj_e"), (12, "uprj_n")):
            p = ps_g.tile([B, 128], f32, tag="pr")
            nc.tensor.matmul(p[:], lhsT=uTb[:], rhs=w(wi), start=True, stop=True)
            srow = sb2.tile([B, 128], bf16, tag=tg)
            nc.vector.tensor_copy(srow[:], p[:])
            uprj.append(srow)
        uWd_row, uWnc_row = uprj

        # ================= EDGE PHASE =================
        stage = None
        for k in range(cfg.NCHE):
            ce = slice(k * CH, (k + 1) * CH)
            ci = slice(k * (CH // 16), (k + 1) * (CH // 16))

            ilo = sb.tile([128, CH // 16], i16, tag="ilo")
            nc.sync.dma_start(ilo[:], t['gsrc_lo'][:, ci])
            ihi = sb.tile([128, CH // 16], i16, tag="ihi")
            nc.sync.dma_start(ihi[:], t['gsrc_hi'][:, ci])
            idl = sb.tile([128, CH // 16], i16, tag="idl")
            nc.sync.dma_start(idl[:], t['gdst'][:, ci])

            g_lo = sb.tile([128, 1, CH], bf16, tag="g_lo")
            nc.gpsimd.dma_gather(g_lo[:], t['Tlo'][:], ilo[:], CH, CH, H,
                                 transpose=True)
            g_hi = sb.tile([128, 1, CH], bf16, tag="g_hi")
            nc.gpsimd.dma_gather(g_hi[:], t['Thi'][:], ihi[:], CH, CH, H,
                                 transpose=True)
            g_dst = sb.tile([128, 1, CH], bf16, tag="g_dst")
            nc.gpsimd.dma_gather(g_dst[:], t['Tloc'][:], idl[:], CH, CH, H,
                                 transpose=True)

            eT_c = sb.tile([128, CH], bf16, tag="eT_c")
            nc.sync.dma_start(eT_c[:], eT_r[:, ce])
            su_c = sb.tile([B, CH], bf16, tag="su_c")
            nc.sync.dma_start(su_c[:], t['S_u'][:, ce])

            h1 = ps_h1.tile([128, CH], f32, tag="h1")
            nc.tensor.matmul(h1[:], lhsT=w(0), rhs=g_lo[:, 0, :], start=True, stop=False)
            nc.tensor.matmul(h1[:], lhsT=w(0), rhs=g_hi[:, 0, :], start=False, stop=False)
            nc.tensor.matmul(h1[:], lhsT=w(1), rhs=g_dst[:, 0, :], start=False, stop=False)
            nc.tensor.matmul(h1[:], lhsT=w(2), rhs=eT_c[:], start=False, stop=False)
            nc.tensor.matmul(h1[:], lhsT=uWd_row[:], rhs=su_c[:], start=False, stop=True)

            rh1 = sb.tile([128, CH], bf16, tag="rh1")
            nc.scalar.activation(rh1[:], h1[:], AF.Relu, bias=bv(0))

            hN = gru(rh1[:], eT_c[:], 4, 1, sb, None, "hN", CH)
            nc.sync.dma_start(eT_w[:, ce], hN[:])

            # transpose to row-form for the scatter
            tpp = ps_tp.tile([128, CH], bf16, tag="tp_b")
            for j in range(NSUB):
                nc.tensor.transpose(tpp[:, j * 128:(j + 1) * 128],
                                    hN[:, j * 128:(j + 1) * 128], ident_b[:])
            kb = k % cfg.CPB
            if kb == 0:
                stage = stg.tile([128, cfg.SCB // 128, H], f32, tag="stage")
            nc.vector.tensor_copy(
                stage[:, kb * NSUB:(kb + 1) * NSUB, :].rearrange("p a b -> p (a b)"),
                tpp[:])
            if kb == cfg.CPB - 1:
                blk = k // cfg.CPB
                isc = sb.tile([128, cfg.SCB // 16], i16, tag="isc")
                nc.sync.dma_start(
                    isc[:], t['sscat'][:, blk * (cfg.SCB // 16):(blk + 1) * (cfg.SCB // 16)])
                nc.gpsimd.dma_scatter_add(agg[:], stage[:], isc[:],
                                          cfg.SCB, cfg.SCB, H)

        # ================= NODE PHASE =================
        for k in range(cfg.NCHN):
            cn = slice(k * CH, (k + 1) * CH)

            agg_rows = sb.tile([128, NSUB, H], f32, tag="agg_rows")
            for j in range(NSUB):
                nc.sync.dma_start(agg_rows[:, j, :],
                                  agg[k * CH + j * 128: k * CH + (j + 1) * 128, :])
            atp = ps_tp.tile([128, CH], f32, tag="tp")
            for j in range(NSUB):
                nc.tensor.transpose(atp[:, j * 128:(j + 1) * 128],
                                    agg_rows[:, j, :], ident_f[:])
            ic = sb.tile([128, CH], bf16, tag="ic")
            nc.sync.dma_start(ic[:], t['invcT'][:, cn])
            aggT = sb.tile([128, CH], bf16, tag="aggT")
            nc.vector.tensor_tensor(aggT[:], atp[:], ic[:], op=ALU.mult)

            snb_c = sb.tile([B, CH], bf16, tag="su_c")
            nc.sync.dma_start(snb_c[:], t['S_nb'][:, cn])

            h1 = ps_h1.tile([128, CH], f32, tag="h1")
            nc.tensor.matmul(h1[:], lhsT=w(10), rhs=xTb[:, cn], start=True, stop=False)
            nc.tensor.matmul(h1[:], lhsT=w(11), rhs=aggT[:], start=False, stop=False)
            nc.tensor.matmul(h1[:], lhsT=uWnc_row[:], rhs=snb_c[:], start=False, stop=True)

            rh1 = sb.tile([128, CH], bf16, tag="rh1")
            nc.scalar.activation(rh1[:], h1[:], AF.Relu, bias=bv(5))

            gru(rh1[:], xTb[:, cn], 13, 6, sb, xT[:, cn], None, CH)
            nc.vector.tensor_copy(xTb[:, cn], xT[:, cn])

            # row-form x for AllGather input, local gather table, graph means
            bmat_c = sb.tile([128, NSUB, B], bf16, tag="bmat_c")
            for j in range(NSUB):
                nc.sync.dma_start(bmat_c[:, j, :],
                                  t['Bmat'][k * CH + j * 128: k * CH + (j + 1) * 128, :])
            bmm = ps_g.tile([128, B], f32, tag="pr")
            for j in range(NSUB):
                xtp = ps_tp.tile([128, 128], bf16, tag="tp_b")
                nc.tensor.transpose(xtp[:], xTb[:, k * CH + j * 128: k * CH + (j + 1) * 128],
                                    ident_b[:])
                xrow = sb.tile([128, 128], bf16, tag="xrow")
                nc.vector.tensor_copy(xrow[:], xtp[:])
                base = k * CH + j * 128
                nrows = max(0, min(128, cfg.NL - base))
                if nrows > 0 and s < cfg.STEPS - 1:
                    nc.sync.dma_start(t['x_shard'][base:base + nrows, :], xrow[:nrows, :])
                    nc.sync.dma_start(t['Tloc'][base:base + nrows, :], xrow[:nrows, :])
                nc.tensor.matmul(bmm[:], lhsT=xrow[:], rhs=bmat_c[:, j, :],
                                 start=(j == 0), stop=(j == NSUB - 1))
            if k == 0:
                nc.vector.tensor_copy(bsum_acc[:], bmm[:])
            else:
                nc.vector.tensor_tensor(bsum_acc[:], bsum_acc[:], bmm[:], op=ALU.add)

        # ================= GLOBAL PHASE =================
        nc.sync.dma_start(t['gsum_in'][:], bsum_acc[:])
        nc.gpsimd.collective_compute(
            "AllReduce", ALU.add, replica_groups=t['rg'],
            ins=[t['gsum_in'][:]], outs=[t['gsum_out'][:]])
        nmF = sb2.tile([128, B], f32, tag="nmF")
        nc.sync.dma_start(nmF[:], t['gsum_out'][:])
        nmT = sb2.tile([128, B], bf16, tag="nmT")
        nc.vector.tensor_copy(nmT[:], nmF[:])

        h1g = ps_h1.tile([128, B], f32, tag="h1")
        nc.tensor.matmul(h1g[:], lhsT=w(19), rhs=uTb[:], start=True, stop=False)
        nc.tensor.matmul(h1g[:], lhsT=w(20), rhs=nmT[:], start=False, stop=True)
        rh1g = sb2.tile([128, B], bf16, tag="rh1g")
        nc.scalar.activation(rh1g[:], h1g[:], AF.Relu, bias=bv(10))

        gru(rh1g[:], uTb[:], 21, 11, sb2, uT[:], None, B)
        nc.vector.tensor_copy(uTb[:], uT[:])

        utp = ps_tp.tile([B, 128], f32, tag="tp")
        nc.tensor.transpose(utp[:], uT[:], ident_f[:])
        urow = sb2.tile([B, 128], f32, tag="urow")
        nc.vector.tensor_copy(urow[:], utp[:])
        nc.sync.dma_start(t['out'][:, s, :], urow[:])

        # ================= AllGather x, rebuild tables =================
        if s < cfg.STEPS - 1:
            nc.gpsimd.collective_compute(
                "AllGather", ALU.bypass, replica_groups=t['rg'],
                ins=[t['x_shard'][:]], outs=[t['x_full'][:]])
            nc.sync.dma_start(t['Tlo'][0:cfg.LO_REAL, :], t['x_full'][0:cfg.LO_REAL, :])
            if cfg.HI_REAL > 0:
                nc.sync.dma_start(t['Thi'][0:cfg.HI_REAL, :],
                                  t['x_full'][cfg.LO_REAL:cfg.N, :])


# ---------------------------------------------------------------- entry point

_CACHE = {}


def kernel(**inputs):
    x = np.asarray(inputs['x'])
    ei = np.asarray(inputs['edge_index'])
    u = np.asarray(inputs['u'])
    cfg = Cfg(N=x.shape[0], E=ei.shape[1], B=u.shape[0], H=x.shape[1], STEPS=3)
    in_maps = host_prepare(cfg, inputs)
    key = (cfg.N, cfg.E, cfg.B, cfg.H, cfg.STEPS, cfg.EPAD)
    if key not in _CACHE:
        _CACHE[key] = build_program(cfg)
    nc = _CACHE[key]
    res = run_bass_kernel_spmd(nc, in_maps, list(range(cfg.NCORES)))
    return np.asarray(res.results[0]["out"], np.float32)


# revision 30
# speedup vs baseline: 1.1387x; 1.1387x over previous
"""Trainium2 Bass kernel for MetaLayer-style GNN (edge/node/global GRU message passing).

Contract: kernel(**inputs) takes the FULL unsharded inputs (np arrays, keys as in
setup_inputs) and returns the FULL output [B, STEPS, H] float32.

Strategy (8 NeuronCores):
- Sort edges by dst, shard nodes into 8 equal contiguous ranges; each core owns all
  edges whose dst is in its range => node aggregation is core-local.
- Per step: edge MLP+GRU (edge-parallel, bf16 matmuls, T-form activations),
  local segment-sum via PE-transpose + dma_scatter_add (fp32, DMA CCE adds),
  node MLP+GRU on local nodes, AllGather of updated x (bf16) to rebuild the
  replicated gather tables, small AllReduce for per-graph node means, replicated
  global MLP+GRU on every core.
- x and u kept resident in fp32 SBUF; MLP second layer folded into GRU input
  weights: gi = relu_h1 @ (Wih@W2).T + (Wih@b2 + bih).
"""

from contextlib import ExitStack

import numpy as np
import ml_dtypes

import concourse.bass as bass
import concourse.bacc as bacc
import concourse.tile as tile
from concourse import mybir
from concourse.bass_utils import run_bass_kernel_spmd
from concourse.masks import make_identity

BF16 = ml_dtypes.bfloat16
AF = mybir.ActivationFunctionType
DT = mybir.dt
ALU = mybir.AluOpType

# ---------------------------------------------------------------- configuration

class Cfg:
    def __init__(self, N=50000, E=500000, B=64, H=128, STEPS=3, NCORES=8,
                 CH=512, SCB=4096):
        assert H == 128
        assert N % NCORES == 0
        self.N, self.E, self.B, self.H, self.STEPS, self.NCORES = N, E, B, H, STEPS, NCORES
        self.CH = CH                      # edge chunk (free dim of f32 PSUM <= 512)
        self.SCB = SCB                    # edges per dma_scatter_add call
        self.NL = N // NCORES             # local nodes
        self.NLP = ((self.NL + CH - 1) // CH) * CH
        self.NCHN = self.NLP // CH        # node chunks
        self.LO_REAL = min(N, 32767)      # x rows in lo table (int16 index limit)
        self.HI_REAL = N - self.LO_REAL
        self.LO_ROWS = self.LO_REAL + 1   # + zero row
        self.HI_ROWS = self.HI_REAL + 1   # + zero row
        self.TLOC_ROWS = self.NL + 1      # + zero row

    def finalize(self, max_shard_edges):
        assert self.SCB % self.CH == 0
        self.EPAD = ((max_shard_edges + self.SCB - 1) // self.SCB) * self.SCB
        self.NCHE = self.EPAD // self.CH  # edge chunks
        self.NSUBS = self.EPAD // 128     # 128-edge subs (one A tile each)
        self.AW = 256                     # aggregation window width (nodes)
        # data-independent window start per sub (aligned 128, clamped)
        self.wstart = []
        for sub in range(self.NSUBS):
            c = (sub + 0.5) * 128 * self.NL / self.EPAD
            w = 128 * int(c // 128) - 64
            w = max(0, min(w, self.NLP - self.AW))
            self.wstart.append(w)
        return self


# ---------------------------------------------------------------- host helpers

def _wrap16(idx, call):
    """Pack indices into the wrapped-16, replicated-128 layout of dma_gather /
    dma_scatter_add: element [p, c*(call//16) + s] = idx[c*call + s*16 + p%16]."""
    total = idx.shape[0]
    assert total % call == 0 and call % 16 == 0
    ncalls = total // call
    w = idx.reshape(ncalls, call // 16, 16)                   # [c, s, lane]
    w = np.transpose(w, (2, 0, 1)).reshape(16, total // 16)   # [lane, c*s]
    w = np.tile(w, (8, 1))                                    # -> 128 partitions
    return np.ascontiguousarray(w.astype(np.int16))


def _onehot(cols_idx, nrows, scale=None, dtype=BF16):
    """[nrows, len(cols_idx)]: out[cols_idx[j], j] = scale_j; idx<0 -> zero col."""
    ncols = cols_idx.shape[0]
    out = np.zeros((nrows, ncols), dtype=np.float32)
    j = np.nonzero(cols_idx >= 0)[0]
    s = np.ones(j.shape[0], np.float32) if scale is None else scale[j]
    out[cols_idx[j], j] = s
    return out.astype(dtype)


def host_prepare(cfg, inputs):
    N, E, B, H = cfg.N, cfg.E, cfg.B, cfg.H
    x = np.asarray(inputs['x'], np.float32)
    edge_index = np.asarray(inputs['edge_index'])
    edge_attr = np.asarray(inputs['edge_attr'], np.float32)
    u = np.asarray(inputs['u'], np.float32)
    batch = np.asarray(inputs['batch']).astype(np.int64)
    src, dst = edge_index[0].astype(np.int64), edge_index[1].astype(np.int64)

    def g(name):
        return np.asarray(inputs[name], np.float32)

    W1, b1 = g('edge_w1'), g('edge_b1')
    W2, b2 = g('edge_w2'), g('edge_b2')
    eWih, eWhh = g('egru_wih'), g('egru_whh')
    eBih, eBhh = g('egru_bih'), g('egru_bhh')
    nW1, nb1 = g('node_w1'), g('node_b1')
    nW2, nb2 = g('node_w2'), g('node_b2')
    nWih, nWhh = g('ngru_wih'), g('ngru_whh')
    nBih, nBhh = g('ngru_bih'), g('ngru_bhh')
    gW1, gb1 = g('glob_w1'), g('glob_b1')
    gW2, gb2 = g('glob_w2'), g('glob_b2')
    gWih, gWhh = g('ggru_wih'), g('ggru_whh')
    gBih, gBhh = g('ggru_bih'), g('ggru_bhh')

    eWih2, eBih2 = eWih @ W2, eWih @ b2 + eBih
    nWih2, nBih2 = nWih @ nW2, nWih @ nb2 + nBih
    gWih2, gBih2 = gWih @ gW2, gWih @ gb2 + gBih

    def gate(Wm, i):
        return Wm[i * H:(i + 1) * H, :].T

    blocks = [
        W1[:, 0:H].T, W1[:, H:2 * H].T, W1[:, 2 * H:3 * H].T, W1[:, 3 * H:4 * H].T,
        gate(eWih2, 0), gate(eWih2, 1), gate(eWih2, 2),
        gate(eWhh, 0), gate(eWhh, 1), gate(eWhh, 2),
        nW1[:, 0:H].T, nW1[:, H:2 * H].T, nW1[:, 2 * H:3 * H].T,
        gate(nWih2, 0), gate(nWih2, 1), gate(nWih2, 2),
        gate(nWhh, 0), gate(nWhh, 1), gate(nWhh, 2),
        gW1[:, 0:H].T, gW1[:, H:2 * H].T,
        gate(gWih2, 0), gate(gWih2, 1), gate(gWih2, 2),
        gate(gWhh, 0), gate(gWhh, 1), gate(gWhh, 2),
    ]
    wpk = np.concatenate([bl.astype(np.float32) for bl in blocks], axis=1).astype(BF16)

    def gb_(v, i):
        return v[i * H:(i + 1) * H]

    bcols = [
        b1, gb_(eBih2, 0) + gb_(eBhh, 0), gb_(eBih2, 1) + gb_(eBhh, 1), gb_(eBhh, 2), gb_(eBih2, 2),
        nb1, gb_(nBih2, 0) + gb_(nBhh, 0), gb_(nBih2, 1) + gb_(nBhh, 1), gb_(nBhh, 2), gb_(nBih2, 2),
        gb1, gb_(gBih2, 0) + gb_(gBhh, 0), gb_(gBih2, 1) + gb_(gBhh, 1), gb_(gBhh, 2), gb_(gBih2, 2),
    ]
    bpk = np.stack(bcols, axis=1).astype(np.float32)

    order = np.argsort(dst, kind='stable')
    ssrc, sdst, sea = src[order], dst[order], edge_attr[order]
    shard_of = sdst // cfg.NL
    counts = np.bincount(shard_of, minlength=cfg.NCORES)
    cfg.finalize(int(counts.max()))

    gcnt = np.bincount(batch, minlength=B).astype(np.float32)
    ginv = 1.0 / np.maximum(gcnt, 1.0)
    ncnt = np.bincount(sdst, minlength=N).astype(np.float32)
    ninv = 1.0 / np.maximum(ncnt, 1.0)
    bsrc_all = batch[ssrc]

    xb = x.astype(BF16)
    in_maps = []
    bounds = np.searchsorted(sdst, np.arange(cfg.NCORES + 1) * cfg.NL)
    for c in range(cfg.NCORES):
        lo_, hi_ = int(bounds[c]), int(bounds[c + 1])
        ne = hi_ - lo_
        npad = cfg.EPAD - ne
        base = c * cfg.NL
        nl, nlp = cfg.NL, cfg.NLP

        # Interleave pads uniformly so slot->node quantile mapping matches the
        # program-uniform window schedule (all-at-end padding would drift).
        pad_slots = np.unique(np.round(np.linspace(0, cfg.EPAD - 1, npad)).astype(np.int64)) \
            if npad > 0 else np.empty(0, np.int64)
        # numerical safety: ensure exactly npad distinct slots
        while pad_slots.shape[0] < npad:
            extra = np.setdiff1d(np.arange(cfg.EPAD), pad_slots)[:npad - pad_slots.shape[0]]
            pad_slots = np.union1d(pad_slots, extra)
        is_pad = np.zeros(cfg.EPAD, bool)
        is_pad[pad_slots] = True
        slot_edge = np.full(cfg.EPAD, -1, np.int64)
        slot_edge[~is_pad] = np.arange(ne)

        def scatter_edges(vals, padval):
            out = np.full(cfg.EPAD, padval, vals.dtype)
            out[~is_pad] = vals
            return out

        csrc = ssrc[lo_:hi_]
        cdst_loc = sdst[lo_:hi_] - base
        cbsrc = bsrc_all[lo_:hi_]

        src_lo = scatter_edges(np.where(csrc < cfg.LO_REAL, csrc, cfg.LO_REAL),
                               np.int64(cfg.LO_REAL))
        src_hi = scatter_edges(np.where(csrc >= cfg.LO_REAL, csrc - cfg.LO_REAL,
                                        cfg.HI_REAL), np.int64(cfg.HI_REAL))
        dst_loc = scatter_edges(cdst_loc, np.int64(cfg.NL))

        # A tiles: per 128-edge sub, one-hot [128, AW] with 1/cnt folded,
        # targeting the sub's data-independent window.
        ws = np.asarray(cfg.wstart)                        # [NSUBS]
        eslot = np.nonzero(~is_pad)[0]                     # slot of real edge i
        rel = cdst_loc - ws[eslot // 128]
        assert rel.min() >= 0 and rel.max() < cfg.AW, \
            f"agg window violated: {rel.min()} {rel.max()}"
        Amat = np.zeros((cfg.NSUBS, 128, cfg.AW), np.float32)
        ninv_loc = ninv[base:base + nl]
        Amat[eslot // 128, eslot % 128, rel] = ninv_loc[cdst_loc]
        Amat = Amat.astype(BF16)

        batch_loc = batch[base:base + nl]
        bl_pad = np.concatenate([batch_loc, np.full(nlp - nl, -1, np.int64)])

        xT0 = np.zeros((128, nlp), np.float32)
        xT0[:, :nl] = x[base:base + nl].T
        eT0 = np.zeros((128, cfg.EPAD), BF16)
        eT0[:, eslot] = sea[lo_:hi_].T.astype(BF16)

        in_maps.append(dict(
            wpk=wpk, bpk=bpk,
            xT0=xT0,
            uT0=np.ascontiguousarray(u.T).astype(np.float32),
            eT0=eT0,
            Tlo0=np.concatenate([xb[:cfg.LO_REAL], np.zeros((1, H), BF16)], axis=0),
            Thi0=np.concatenate([xb[cfg.LO_REAL:], np.zeros((1, H), BF16)], axis=0),
            Tloc0=np.concatenate([xb[base:base + nl], np.zeros((1, H), BF16)], axis=0),
            gsrc_lo=_wrap16(src_lo, cfg.CH),
            gsrc_hi=_wrap16(src_hi, cfg.CH),
            gdst=_wrap16(dst_loc, cfg.CH),
            Amat=Amat,
            S_u=_onehot(scatter_edges(cbsrc, np.int64(-1)), B),
            S_nb=_onehot(bl_pad, B),
            Bmat=np.ascontiguousarray(
                _onehot(bl_pad, B, scale=ginv[np.clip(bl_pad, 0, B - 1)]).T),
        ))
    return in_maps


# ---------------------------------------------------------------- device program

def build_program(cfg):
    nc = bacc.Bacc("TRN2", target_bir_lowering=False, debug=False,
                   num_devices=cfg.NCORES, num_swdge_queues=4)
    H, B, CH = cfg.H, cfg.B, cfg.CH
    NW = 27
    f32, bf16, i16 = DT.float32, DT.bfloat16, DT.int16

    def din(name, shape, dt):
        return nc.dram_tensor(name, shape, dt, kind="ExternalInput").ap()

    t = {}
    t['wpk'] = din("wpk", [128, NW * 128], bf16)
    t['bpk'] = din("bpk", [128, 15], f32)
    t['xT0'] = din("xT0", [128, cfg.NLP], f32)
    t['uT0'] = din("uT0", [128, B], f32)
    t['eT0'] = din("eT0", [128, cfg.EPAD], bf16)
    t['Tlo0'] = din("Tlo0", [cfg.LO_ROWS, H], bf16)
    t['Thi0'] = din("Thi0", [cfg.HI_ROWS, H], bf16)
    t['Tloc0'] = din("Tloc0", [cfg.TLOC_ROWS, H], bf16)
    t['gsrc_lo'] = din("gsrc_lo", [128, cfg.EPAD // 16], i16)
    t['gsrc_hi'] = din("gsrc_hi", [128, cfg.EPAD // 16], i16)
    t['gdst'] = din("gdst", [128, cfg.EPAD // 16], i16)
    t['Amat'] = din("Amat", [cfg.NSUBS, 128, cfg.AW], bf16)
    t['S_u'] = din("S_u", [B, cfg.EPAD], bf16)
    t['S_nb'] = din("S_nb", [B, cfg.NLP], bf16)
    t['Bmat'] = din("Bmat", [cfg.NLP, B], bf16)

    t['out'] = nc.dram_tensor("out", [B, cfg.STEPS, H], f32, kind="ExternalOutput").ap()

    t['eTd'] = [nc.dram_tensor(f"eTd{i}", [128, cfg.EPAD], bf16).ap() for i in range(2)]
    t['Tlo'] = nc.dram_tensor("Tlo", [cfg.LO_ROWS, H], bf16).ap()
    t['Thi'] = nc.dram_tensor("Thi", [cfg.HI_ROWS, H], bf16).ap()
    t['Tloc'] = nc.dram_tensor("Tloc", [cfg.TLOC_ROWS, H], bf16).ap()
    t['x_shard'] = nc.dram_tensor("x_shard", [cfg.NL, H], bf16).ap()
    t['x_full'] = nc.dram_tensor("x_full", [cfg.N, H], bf16, addr_space="Shared").ap()
    t['gsum_in'] = nc.dram_tensor("gsum_in", [128, B], f32).ap()
    t['gsum_out'] = nc.dram_tensor("gsum_out", [128, B], f32, addr_space="Shared").ap()
    t['rg'] = [list(range(cfg.NCORES))]

    with ExitStack() as ctx:
        tc = ctx.enter_context(tile.TileContext(nc))
        _emit(nc, tc, ctx, cfg, t)
    nc.compile()
    return nc


def _emit(nc, tc, ctx, cfg, t):
    H, B, CH = cfg.H, cfg.B, cfg.CH
    f32, bf16, i16 = DT.float32, DT.bfloat16, DT.int16
    NSUB = CH // 128

    perm = ctx.enter_context(tc.tile_pool(name="perm", bufs=1))
    sb = ctx.enter_context(tc.tile_pool(name="sb", bufs=2))
    sb2 = ctx.enter_context(tc.tile_pool(name="sb2", bufs=2))
    ps_h1 = ctx.enter_context(tc.tile_pool(name="ps_h1", bufs=2, space="PSUM"))
    ps_g = ctx.enter_context(tc.tile_pool(name="ps_g", bufs=1, space="PSUM"))
    ps_tp = ctx.enter_context(tc.tile_pool(name="ps_tp", bufs=1, space="PSUM"))

    # ---------------- persistent SBUF state
    W = perm.tile([128, 27 * 128], bf16)
    nc.sync.dma_start(W[:], t['wpk'][:])

    def w(i):
        return W[:, i * 128:(i + 1) * 128]

    bias = perm.tile([128, 15], f32)
    nc.sync.dma_start(bias[:], t['bpk'][:])

    def bv(i):
        return bias[:, i:i + 1]

    xT = perm.tile([128, cfg.NLP], f32)
    nc.sync.dma_start(xT[:], t['xT0'][:])
    xTb = perm.tile([128, cfg.NLP], bf16)
    nc.vector.tensor_copy(xTb[:], xT[:])

    uT = perm.tile([128, B], f32)
    nc.sync.dma_start(uT[:], t['uT0'][:])
    uTb = perm.tile([128, B], bf16)
    nc.vector.tensor_copy(uTb[:], uT[:])

    bsum_acc = perm.tile([128, B], f32)
    aggT = perm.tile([128, cfg.NLP], f32)     # resident aggregation accumulator

    ident_f = perm.tile([128, 128], f32)
    make_identity(nc, ident_f[:])
    ident_b = perm.tile([128, 128], bf16)
    nc.vector.tensor_copy(ident_b[:], ident_f[:])

    # ---------------- init DRAM state
    nc.sync.dma_start(t['eTd'][0][:], t['eT0'][:])
    nc.sync.dma_start(t['Tlo'][:], t['Tlo0'][:])
    nc.sync.dma_start(t['Thi'][:], t['Thi0'][:])
    nc.sync.dma_start(t['Tloc'][:], t['Tloc0'][:])

    def gru(xiT, hTb, wb, bb, pool, h_f32, out_tag, FD):
        """GRU tail: xiT bf16 [128,FD] (input through W2 fold), hTb bf16 [128,FD].
        If h_f32 given: blend in f32 in-place there and return None; else return
        a bf16 tile. wb: base index of Wih2 r,z,g then Whh r,z,g. bb: bias base."""
        pr = ps_g.tile([128, FD], f32, tag="pr")
        nc.tensor.matmul(pr[:], lhsT=w(wb + 0), rhs=xiT, start=True, stop=False)
        nc.tensor.matmul(pr[:], lhsT=w(wb + 3), rhs=hTb, start=False, stop=True)
        pz = ps_g.tile([128, FD], f32, tag="pz")
        nc.tensor.matmul(pz[:], lhsT=w(wb + 1), rhs=xiT, start=True, stop=False)
        nc.tensor.matmul(pz[:], lhsT=w(wb + 4), rhs=hTb, start=False, stop=True)
        pig = ps_g.tile([128, FD], f32, tag="pig")
        nc.tensor.matmul(pig[:], lhsT=w(wb + 2), rhs=xiT, start=True, stop=True)
        phg = ps_g.tile([128, FD], f32, tag="phg")
        nc.tensor.matmul(phg[:], lhsT=w(wb + 5), rhs=hTb, start=True, stop=True)

        r = pool.tile([128, FD], bf16, tag="r")
        nc.scalar.activation(r[:], pr[:], AF.Sigmoid, bias=bv(bb + 0))
        z = pool.tile([128, FD], bf16, tag="z")
        nc.scalar.activation(z[:], pz[:], AF.Sigmoid, bias=bv(bb + 1))
        hg = pool.tile([128, FD], bf16, tag="hg")
        nc.scalar.activation(hg[:], phg[:], AF.Identity, bias=bv(bb + 2))
        tm = pool.tile([128, FD], f32, tag="tm")
        nc.vector.tensor_tensor(tm[:], r[:], hg[:], op=ALU.mult)
        sp = pool.tile([128, FD], f32, tag="sp")
        nc.vector.tensor_tensor(sp[:], tm[:], pig[:], op=ALU.add)
        n = pool.tile([128, FD], bf16, tag="n")
        nc.scalar.activation(n[:], sp[:], AF.Tanh, bias=bv(bb + 3))

        hold = h_f32 if h_f32 is not None else hTb
        d = pool.tile([128, FD], f32, tag="d")
        nc.vector.tensor_tensor(d[:], hold, n[:], op=ALU.subtract)
        m = pool.tile([128, FD], f32, tag="m")
        nc.vector.tensor_tensor(m[:], z[:], d[:], op=ALU.mult)
        if h_f32 is not None:
            nc.vector.tensor_tensor(h_f32, n[:], m[:], op=ALU.add)
            return None
        hN = pool.tile([128, FD], bf16, tag=out_tag)
        nc.vector.tensor_tensor(hN[:], n[:], m[:], op=ALU.add)
        return hN

    aw_ps = None
    for s in range(cfg.STEPS):
        eT_r, eT_w = t['eTd'][s % 2], t['eTd'][(s + 1) % 2]
        nc.vector.memset(aggT[:], 0.0)

        # per-step u projections: uWd_row = u @ W1d.T ; uWnc_row = u @ Wn1c.T
        uprj = []
        for wi, tg in ((3, "uprj_e"), (12, "uprj_n")):
            p = ps_g.tile([B, 128], f32, tag="pr")
            nc.tensor.matmul(p[:], lhsT=uTb[:], rhs=w(wi), start=True, stop=True)
            srow = sb2.tile([B, 128], bf16, tag=tg)
            nc.vector.tensor_copy(srow[:], p[:])
            uprj.append(srow)
        uWd_row, uWnc_row = uprj

        # ================= EDGE PHASE =================
        stage = None
        for k in range(cfg.NCHE):
            ce = slice(k * CH, (k + 1) * CH)
            ci = slice(k * (CH // 16), (k + 1) * (CH // 16))

            ilo = sb.tile([128, CH // 16], i16, tag="ilo")
            nc.sync.dma_start(ilo[:], t['gsrc_lo'][:, ci])
            ihi = sb.tile([128, CH // 16], i16, tag="ihi")
            nc.sync.dma_start(ihi[:], t['gsrc_hi'][:, ci])
            idl = sb.tile([128, CH // 16], i16, tag="idl")
            nc.sync.dma_start(idl[:], t['gdst'][:, ci])

            g_lo = sb.tile([128, 1, CH], bf16, tag="g_lo")
            nc.gpsimd.dma_gather(g_lo[:], t['Tlo'][:], ilo[:], CH, CH, H,
                                 transpose=True)
            g_hi = sb.tile([128, 1, CH], bf16, tag="g_hi")
            nc.gpsimd.dma_gather(g_hi[:], t['Thi'][:], ihi[:], CH, CH, H,
                                 transpose=True)
            g_dst = sb.tile([128, 1, CH], bf16, tag="g_dst")
            nc.gpsimd.dma_gather(g_dst[:], t['Tloc'][:], idl[:], CH, CH, H,
                                 transpose=True)

            eT_c = sb.tile([128, CH], bf16, tag="eT_c")
            nc.sync.dma_start(eT_c[:], eT_r[:, ce])
            su_c = sb.tile([B, CH], bf16, tag="su_c")
            nc.sync.dma_start(su_c[:], t['S_u'][:, ce])

            h1 = ps_h1.tile([128, CH], f32, tag="h1")
            nc.tensor.matmul(h1[:], lhsT=w(0), rhs=g_lo[:, 0, :], start=True, stop=False)
            nc.tensor.matmul(h1[:], lhsT=w(0), rhs=g_hi[:, 0, :], start=False, stop=False)
            nc.tensor.matmul(h1[:], lhsT=w(1), rhs=g_dst[:, 0, :], start=False, stop=False)
            nc.tensor.matmul(h1[:], lhsT=w(2), rhs=eT_c[:], start=False, stop=False)
            nc.tensor.matmul(h1[:], lhsT=uWd_row[:], rhs=su_c[:], start=False, stop=True)

            rh1 = sb.tile([128, CH], bf16, tag="rh1")
            nc.scalar.activation(rh1[:], h1[:], AF.Relu, bias=bv(0))

            hN = gru(rh1[:], eT_c[:], 4, 1, sb, None, "hN", CH)
            nc.sync.dma_start(eT_w[:, ce], hN[:])

            # transpose to row-form, then aggregate via one-hot A matmuls
            tpp = ps_tp.tile([128, CH], bf16, tag="tp_b")
            for j in range(NSUB):
                nc.tensor.transpose(tpp[:, j * 128:(j + 1) * 128],
                                    hN[:, j * 128:(j + 1) * 128], ident_b[:])
            erow = sb.tile([128, CH], bf16, tag="erow")
            nc.vector.tensor_copy(erow[:], tpp[:])
            for j in range(NSUB):
                gs = k * NSUB + j
                wb = cfg.wstart[gs]
                first = (gs == 0) or (cfg.wstart[gs - 1] != wb)
                last = (gs == cfg.NSUBS - 1) or (cfg.wstart[gs + 1] != wb)
                atile = sb.tile([128, cfg.AW], bf16, tag="atile")
                nc.sync.dma_start(atile[:], t['Amat'][gs, :, :])
                if first:
                    aw_ps = ps_tp.tile([128, cfg.AW], f32, tag="aw")
                nc.tensor.matmul(aw_ps[:], lhsT=erow[:, j * 128:(j + 1) * 128],
                                 rhs=atile[:], start=first, stop=last)
                if last:
                    nc.vector.tensor_tensor(aggT[:, wb:wb + cfg.AW],
                                            aggT[:, wb:wb + cfg.AW],
                                            aw_ps[:], op=ALU.add)

        # ================= NODE PHASE =================
        for k in range(cfg.NCHN):
            cn = slice(k * CH, (k + 1) * CH)

            aggTb = sb.tile([128, CH], bf16, tag="aggTb")
            nc.vector.tensor_copy(aggTb[:], aggT[:, cn])

            snb_c = sb.tile([B, CH], bf16, tag="su_c")
            nc.sync.dma_start(snb_c[:], t['S_nb'][:, cn])

            h1 = ps_h1.tile([128, CH], f32, tag="h1")
            nc.tensor.matmul(h1[:], lhsT=w(10), rhs=xTb[:, cn], start=True, stop=False)
            nc.tensor.matmul(h1[:], lhsT=w(11), rhs=aggTb[:], start=False, stop=False)
            nc.tensor.matmul(h1[:], lhsT=uWnc_row[:], rhs=snb_c[:], start=False, stop=True)

            rh1 = sb.tile([128, CH], bf16, tag="rh1")
            nc.scalar.activation(rh1[:], h1[:], AF.Relu, bias=bv(5))

            gru(rh1[:], xTb[:, cn], 13, 6, sb, xT[:, cn], None, CH)
            nc.vector.tensor_copy(xTb[:, cn], xT[:, cn])

            # row-form x for AllGather input, local gather table, graph means
            bmat_c = sb.tile([128, NSUB, B], bf16, tag="bmat_c")
            for j in range(NSUB):
                nc.sync.dma_start(bmat_c[:, j, :],
                                  t['Bmat'][k * CH + j * 128: k * CH + (j + 1) * 128, :])
            bmm = ps_g.tile([128, B], f32, tag="pr")
            for j in range(NSUB):
                xtp = ps_tp.tile([128, 128], bf16, tag="tp_b")
                nc.tensor.transpose(xtp[:], xTb[:, k * CH + j * 128: k * CH + (j + 1) * 128],
                                    ident_b[:])
                xrow = sb.tile([128, 128], bf16, tag="xrow")
                nc.vector.tensor_copy(xrow[:], xtp[:])
                base = k * CH + j * 128
                nrows = max(0, min(128, cfg.NL - base))
                if nrows > 0 and s < cfg.STEPS - 1:
                    nc.sync.dma_start(t['x_shard'][base:base + nrows, :], xrow[:nrows, :])
                    nc.sync.dma_start(t['Tloc'][base:base + nrows, :], xrow[:nrows, :])
                nc.tensor.matmul(bmm[:], lhsT=xrow[:], rhs=bmat_c[:, j, :],
                                 start=(j == 0), stop=(j == NSUB - 1))
            if k == 0:
                nc.vector.tensor_copy(bsum_acc[:], bmm[:])
            else:
                nc.vector.tensor_tensor(bsum_acc[:], bsum_acc[:], bmm[:], op=ALU.add)

        # ================= GLOBAL PHASE =================
        nc.sync.dma_start(t['gsum_in'][:], bsum_acc[:])
        nc.gpsimd.collective_compute(
            "AllReduce", ALU.add, replica_groups=t['rg'],
            ins=[t['gsum_in'][:]], outs=[t['gsum_out'][:]])
        nmF = sb2.tile([128, B], f32, tag="nmF")
        nc.sync.dma_start(nmF[:], t['gsum_out'][:])
        nmT = sb2.tile([128, B], bf16, tag="nmT")
        nc.vector.tensor_copy(nmT[:], nmF[:])

        h1g = ps_h1.tile([128, B], f32, tag="h1")
        nc.tensor.matmul(h1g[:], lhsT=w(19), rhs=uTb[:], start=True, stop=False)
        nc.tensor.matmul(h1g[:], lhsT=w(20), rhs=nmT[:], start=False, stop=True)
        rh1g = sb2.tile([128, B], bf16, tag="rh1g")
        nc.scalar.activation(rh1g[:], h1g[:], AF.Relu, bias=bv(10))

        gru(rh1g[:], uTb[:], 21, 11, sb2, uT[:], None, B)
        nc.vector.tensor_copy(uTb[:], uT[:])

        utp = ps_tp.tile([B, 128], f32, tag="aw")
        nc.tensor.transpose(utp[:], uT[:], ident_f[:])
        urow = sb2.tile([B, 128], f32, tag="urow")
        nc.vector.tensor_copy(urow[:], utp[:])
        nc.sync.dma_start(t['out'][:, s, :], urow[:])

        # ================= AllGather x, rebuild tables =================
        if s < cfg.STEPS - 1:
            nc.gpsimd.collective_compute(
                "AllGather", ALU.bypass, replica_groups=t['rg'],
                ins=[t['x_shard'][:]], outs=[t['x_full'][:]])
            nc.sync.dma_start(t['Tlo'][0:cfg.LO_REAL, :], t['x_full'][0:cfg.LO_REAL, :])
            if cfg.HI_REAL > 0:
                nc.sync.dma_start(t['Thi'][0:cfg.HI_REAL, :],
                                  t['x_full'][cfg.LO_REAL:cfg.N, :])


# ---------------------------------------------------------------- entry point

_CACHE = {}


def kernel(**inputs):
    x = np.asarray(inputs['x'])
    ei = np.asarray(inputs['edge_index'])
    u = np.asarray(inputs['u'])
    cfg = Cfg(N=x.shape[0], E=ei.shape[1], B=u.shape[0], H=x.shape[1], STEPS=3)
    in_maps = host_prepare(cfg, inputs)
    key = (cfg.N, cfg.E, cfg.B, cfg.H, cfg.STEPS, cfg.EPAD)
    if key not in _CACHE:
        _CACHE[key] = build_program(cfg)
    nc = _CACHE[key]
    res = run_bass_kernel_spmd(nc, in_maps, list(range(cfg.NCORES)))
    return np.asarray(res.results[0]["out"], np.float32)


# revision 32
# speedup vs baseline: 1.3167x; 1.1563x over previous
"""Trainium2 Bass kernel for MetaLayer-style GNN (edge/node/global GRU message passing).

Contract: kernel(**inputs) takes the FULL unsharded inputs (np arrays, keys as in
setup_inputs) and returns the FULL output [B, STEPS, H] float32.

Strategy (8 NeuronCores):
- Sort edges by dst, shard nodes into 8 equal contiguous ranges; each core owns all
  edges whose dst is in its range => node aggregation is core-local.
- Per step: edge MLP+GRU (edge-parallel, bf16 matmuls, T-form activations),
  local segment-sum via PE-transpose + dma_scatter_add (fp32, DMA CCE adds),
  node MLP+GRU on local nodes, AllGather of updated x (bf16) to rebuild the
  replicated gather tables, small AllReduce for per-graph node means, replicated
  global MLP+GRU on every core.
- x and u kept resident in fp32 SBUF; MLP second layer folded into GRU input
  weights: gi = relu_h1 @ (Wih@W2).T + (Wih@b2 + bih).
"""

from contextlib import ExitStack

import numpy as np
import ml_dtypes

import concourse.bass as bass
import concourse.bacc as bacc
import concourse.tile as tile
from concourse import mybir
from concourse.bass_utils import run_bass_kernel_spmd
from concourse.masks import make_identity

BF16 = ml_dtypes.bfloat16
AF = mybir.ActivationFunctionType
DT = mybir.dt
ALU = mybir.AluOpType

# ---------------------------------------------------------------- configuration

class Cfg:
    def __init__(self, N=50000, E=500000, B=64, H=128, STEPS=3, NCORES=8,
                 CH=512, SCB=4096):
        assert H == 128
        assert N % NCORES == 0
        self.N, self.E, self.B, self.H, self.STEPS, self.NCORES = N, E, B, H, STEPS, NCORES
        self.CH = CH                      # edge chunk (free dim of f32 PSUM <= 512)
        self.SCB = SCB                    # edges per dma_scatter_add call
        self.NL = N // NCORES             # local nodes
        self.NLP = ((self.NL + CH - 1) // CH) * CH
        self.NCHN = self.NLP // CH        # node chunks
        self.LO_REAL = min(N, 32767)      # x rows in lo table (int16 index limit)
        self.HI_REAL = N - self.LO_REAL
        self.LO_ROWS = self.LO_REAL + 1   # + zero row
        self.HI_ROWS = self.HI_REAL + 1   # + zero row
        self.TLOC_ROWS = self.NL + 1      # + zero row

    def finalize(self, max_shard_edges):
        assert self.SCB % self.CH == 0
        self.EPAD = ((max_shard_edges + self.SCB - 1) // self.SCB) * self.SCB
        self.NCHE = self.EPAD // self.CH  # edge chunks
        self.NSUBS = self.EPAD // 128     # 128-edge subs (one A tile each)
        self.AW = 256                     # aggregation window width (nodes)
        # data-independent window start per sub (aligned 128, clamped)
        self.wstart = []
        for sub in range(self.NSUBS):
            c = (sub + 0.5) * 128 * self.NL / self.EPAD
            w = 128 * int(c // 128) - 64
            w = max(0, min(w, self.NLP - self.AW))
            self.wstart.append(w)
        return self


# ---------------------------------------------------------------- host helpers

def _wrap16(idx, call):
    """Pack indices into the wrapped-16, replicated-128 layout of dma_gather /
    dma_scatter_add: element [p, c*(call//16) + s] = idx[c*call + s*16 + p%16]."""
    total = idx.shape[0]
    assert total % call == 0 and call % 16 == 0
    ncalls = total // call
    w = idx.reshape(ncalls, call // 16, 16)                   # [c, s, lane]
    w = np.transpose(w, (2, 0, 1)).reshape(16, total // 16)   # [lane, c*s]
    w = np.tile(w, (8, 1))                                    # -> 128 partitions
    return np.ascontiguousarray(w.astype(np.int16))


def _onehot(cols_idx, nrows, scale=None, dtype=BF16):
    """[nrows, len(cols_idx)]: out[cols_idx[j], j] = scale_j; idx<0 -> zero col."""
    ncols = cols_idx.shape[0]
    out = np.zeros((nrows, ncols), dtype=np.float32)
    j = np.nonzero(cols_idx >= 0)[0]
    s = np.ones(j.shape[0], np.float32) if scale is None else scale[j]
    out[cols_idx[j], j] = s
    return out.astype(dtype)


def host_prepare(cfg, inputs):
    N, E, B, H = cfg.N, cfg.E, cfg.B, cfg.H
    x = np.asarray(inputs['x'], np.float32)
    edge_index = np.asarray(inputs['edge_index'])
    edge_attr = np.asarray(inputs['edge_attr'], np.float32)
    u = np.asarray(inputs['u'], np.float32)
    batch = np.asarray(inputs['batch']).astype(np.int64)
    src, dst = edge_index[0].astype(np.int64), edge_index[1].astype(np.int64)

    def g(name):
        return np.asarray(inputs[name], np.float32)

    W1, b1 = g('edge_w1'), g('edge_b1')
    W2, b2 = g('edge_w2'), g('edge_b2')
    eWih, eWhh = g('egru_wih'), g('egru_whh')
    eBih, eBhh = g('egru_bih'), g('egru_bhh')
    nW1, nb1 = g('node_w1'), g('node_b1')
    nW2, nb2 = g('node_w2'), g('node_b2')
    nWih, nWhh = g('ngru_wih'), g('ngru_whh')
    nBih, nBhh = g('ngru_bih'), g('ngru_bhh')
    gW1, gb1 = g('glob_w1'), g('glob_b1')
    gW2, gb2 = g('glob_w2'), g('glob_b2')
    gWih, gWhh = g('ggru_wih'), g('ggru_whh')
    gBih, gBhh = g('ggru_bih'), g('ggru_bhh')

    eWih2, eBih2 = eWih @ W2, eWih @ b2 + eBih
    nWih2, nBih2 = nWih @ nW2, nWih @ nb2 + nBih
    gWih2, gBih2 = gWih @ gW2, gWih @ gb2 + gBih

    def gate(Wm, i):
        return Wm[i * H:(i + 1) * H, :].T

    blocks = [
        W1[:, 0:H].T, W1[:, H:2 * H].T, W1[:, 2 * H:3 * H].T, W1[:, 3 * H:4 * H].T,
        gate(eWih2, 0), gate(eWih2, 1), gate(eWih2, 2),
        gate(eWhh, 0), gate(eWhh, 1), gate(eWhh, 2),
        nW1[:, 0:H].T, nW1[:, H:2 * H].T, nW1[:, 2 * H:3 * H].T,
        gate(nWih2, 0), gate(nWih2, 1), gate(nWih2, 2),
        gate(nWhh, 0), gate(nWhh, 1), gate(nWhh, 2),
        gW1[:, 0:H].T, gW1[:, H:2 * H].T,
        gate(gWih2, 0), gate(gWih2, 1), gate(gWih2, 2),
        gate(gWhh, 0), gate(gWhh, 1), gate(gWhh, 2),
    ]
    wpk = np.concatenate([bl.astype(np.float32) for bl in blocks], axis=1).astype(BF16)

    def gb_(v, i):
        return v[i * H:(i + 1) * H]

    bcols = [
        b1, gb_(eBih2, 0) + gb_(eBhh, 0), gb_(eBih2, 1) + gb_(eBhh, 1), gb_(eBhh, 2), gb_(eBih2, 2),
        nb1, gb_(nBih2, 0) + gb_(nBhh, 0), gb_(nBih2, 1) + gb_(nBhh, 1), gb_(nBhh, 2), gb_(nBih2, 2),
        gb1, gb_(gBih2, 0) + gb_(gBhh, 0), gb_(gBih2, 1) + gb_(gBhh, 1), gb_(gBhh, 2), gb_(gBih2, 2),
    ]
    bpk = np.stack(bcols, axis=1).astype(np.float32)

    order = np.argsort(dst, kind='stable')
    ssrc, sdst, sea = src[order], dst[order], edge_attr[order]
    shard_of = sdst // cfg.NL
    counts = np.bincount(shard_of, minlength=cfg.NCORES)
    cfg.finalize(int(counts.max()))

    gcnt = np.bincount(batch, minlength=B).astype(np.float32)
    ginv = 1.0 / np.maximum(gcnt, 1.0)
    ncnt = np.bincount(sdst, minlength=N).astype(np.float32)
    ninv = 1.0 / np.maximum(ncnt, 1.0)
    bsrc_all = batch[ssrc]

    xb = x.astype(BF16)
    in_maps = []
    bounds = np.searchsorted(sdst, np.arange(cfg.NCORES + 1) * cfg.NL)
    for c in range(cfg.NCORES):
        lo_, hi_ = int(bounds[c]), int(bounds[c + 1])
        ne = hi_ - lo_
        npad = cfg.EPAD - ne
        base = c * cfg.NL
        nl, nlp = cfg.NL, cfg.NLP

        # Interleave pads uniformly so slot->node quantile mapping matches the
        # program-uniform window schedule (all-at-end padding would drift).
        pad_slots = np.unique(np.round(np.linspace(0, cfg.EPAD - 1, npad)).astype(np.int64)) \
            if npad > 0 else np.empty(0, np.int64)
        # numerical safety: ensure exactly npad distinct slots
        while pad_slots.shape[0] < npad:
            extra = np.setdiff1d(np.arange(cfg.EPAD), pad_slots)[:npad - pad_slots.shape[0]]
            pad_slots = np.union1d(pad_slots, extra)
        is_pad = np.zeros(cfg.EPAD, bool)
        is_pad[pad_slots] = True
        slot_edge = np.full(cfg.EPAD, -1, np.int64)
        slot_edge[~is_pad] = np.arange(ne)

        def scatter_edges(vals, padval):
            out = np.full(cfg.EPAD, padval, vals.dtype)
            out[~is_pad] = vals
            return out

        csrc = ssrc[lo_:hi_]
        cdst_loc = sdst[lo_:hi_] - base
        cbsrc = bsrc_all[lo_:hi_]

        src_lo = scatter_edges(np.where(csrc < cfg.LO_REAL, csrc, cfg.LO_REAL),
                               np.int64(cfg.LO_REAL))
        src_hi = scatter_edges(np.where(csrc >= cfg.LO_REAL, csrc - cfg.LO_REAL,
                                        cfg.HI_REAL), np.int64(cfg.HI_REAL))
        dst_loc = scatter_edges(cdst_loc, np.int64(cfg.NL))

        # A tiles: per 128-edge sub, one-hot [128, AW] with 1/cnt folded,
        # targeting the sub's data-independent window.
        ws = np.asarray(cfg.wstart)                        # [NSUBS]
        eslot = np.nonzero(~is_pad)[0]                     # slot of real edge i
        rel = cdst_loc - ws[eslot // 128]
        assert rel.min() >= 0 and rel.max() < cfg.AW, \
            f"agg window violated: {rel.min()} {rel.max()}"
        Amat = np.zeros((cfg.NSUBS, 128, cfg.AW), np.float32)
        ninv_loc = ninv[base:base + nl]
        Amat[eslot // 128, eslot % 128, rel] = ninv_loc[cdst_loc]
        Amat = Amat.astype(BF16)

        batch_loc = batch[base:base + nl]
        bl_pad = np.concatenate([batch_loc, np.full(nlp - nl, -1, np.int64)])

        xT0 = np.zeros((128, nlp), np.float32)
        xT0[:, :nl] = x[base:base + nl].T
        eT0 = np.zeros((128, cfg.EPAD), BF16)
        eT0[:, eslot] = sea[lo_:hi_].T.astype(BF16)

        in_maps.append(dict(
            wpk=wpk, bpk=bpk,
            xT0=xT0,
            uT0=np.ascontiguousarray(u.T).astype(np.float32),
            eT0=eT0,
            Tlo0=np.concatenate([xb[:cfg.LO_REAL], np.zeros((1, H), BF16)], axis=0),
            Thi0=np.concatenate([xb[cfg.LO_REAL:], np.zeros((1, H), BF16)], axis=0),
            Tloc0=np.concatenate([xb[base:base + nl], np.zeros((1, H), BF16)], axis=0),
            gsrc_lo=_wrap16(src_lo, cfg.CH),
            gsrc_hi=_wrap16(src_hi, cfg.CH),
            gdst=_wrap16(dst_loc, cfg.CH),
            Amat=Amat,
            S_u=_onehot(scatter_edges(cbsrc, np.int64(-1)), B),
            S_nb=_onehot(bl_pad, B),
            Bmat=np.ascontiguousarray(
                _onehot(bl_pad, B, scale=ginv[np.clip(bl_pad, 0, B - 1)]).T),
        ))
    return in_maps


# ---------------------------------------------------------------- device program

def build_program(cfg):
    nc = bacc.Bacc("TRN2", target_bir_lowering=False, debug=False,
                   num_devices=cfg.NCORES, num_swdge_queues=4)
    H, B, CH = cfg.H, cfg.B, cfg.CH
    NW = 27
    f32, bf16, i16 = DT.float32, DT.bfloat16, DT.int16

    def din(name, shape, dt):
        return nc.dram_tensor(name, shape, dt, kind="ExternalInput").ap()

    t = {}
    t['wpk'] = din("wpk", [128, NW * 128], bf16)
    t['bpk'] = din("bpk", [128, 15], f32)
    t['xT0'] = din("xT0", [128, cfg.NLP], f32)
    t['uT0'] = din("uT0", [128, B], f32)
    t['eT0'] = din("eT0", [128, cfg.EPAD], bf16)
    t['Tlo0'] = din("Tlo0", [cfg.LO_ROWS, H], bf16)
    t['Thi0'] = din("Thi0", [cfg.HI_ROWS, H], bf16)
    t['Tloc0'] = din("Tloc0", [cfg.TLOC_ROWS, H], bf16)
    t['gsrc_lo'] = din("gsrc_lo", [128, cfg.EPAD // 16], i16)
    t['gsrc_hi'] = din("gsrc_hi", [128, cfg.EPAD // 16], i16)
    t['gdst'] = din("gdst", [128, cfg.EPAD // 16], i16)
    t['Amat'] = din("Amat", [cfg.NSUBS, 128, cfg.AW], bf16)
    t['S_u'] = din("S_u", [B, cfg.EPAD], bf16)
    t['S_nb'] = din("S_nb", [B, cfg.NLP], bf16)
    t['Bmat'] = din("Bmat", [cfg.NLP, B], bf16)

    t['out'] = nc.dram_tensor("out", [B, cfg.STEPS, H], f32, kind="ExternalOutput").ap()

    t['eTd'] = [nc.dram_tensor(f"eTd{i}", [128, cfg.EPAD], bf16).ap() for i in range(2)]
    t['Tlo'] = nc.dram_tensor("Tlo", [cfg.LO_ROWS, H], bf16).ap()
    t['Thi'] = nc.dram_tensor("Thi", [cfg.HI_ROWS, H], bf16).ap()
    t['Tloc'] = nc.dram_tensor("Tloc", [cfg.TLOC_ROWS, H], bf16).ap()
    t['x_shard'] = nc.dram_tensor("x_shard", [cfg.NL, H], bf16).ap()
    t['x_full'] = nc.dram_tensor("x_full", [cfg.N, H], bf16, addr_space="Shared").ap()
    t['gsum_in'] = nc.dram_tensor("gsum_in", [128, B], f32).ap()
    t['gsum_out'] = nc.dram_tensor("gsum_out", [128, B], f32, addr_space="Shared").ap()
    t['rg'] = [list(range(cfg.NCORES))]

    with ExitStack() as ctx:
        tc = ctx.enter_context(tile.TileContext(nc))
        _emit(nc, tc, ctx, cfg, t)
    nc.compile()
    return nc


def _emit(nc, tc, ctx, cfg, t):
    H, B, CH = cfg.H, cfg.B, cfg.CH
    f32, bf16, i16 = DT.float32, DT.bfloat16, DT.int16
    NSUB = CH // 128

    perm = ctx.enter_context(tc.tile_pool(name="perm", bufs=1))
    sb = ctx.enter_context(tc.tile_pool(name="sb", bufs=2))
    sb2 = ctx.enter_context(tc.tile_pool(name="sb2", bufs=2))
    ps_h1 = ctx.enter_context(tc.tile_pool(name="ps_h1", bufs=2, space="PSUM"))
    ps_g = ctx.enter_context(tc.tile_pool(name="ps_g", bufs=1, space="PSUM"))
    ps_tp = ctx.enter_context(tc.tile_pool(name="ps_tp", bufs=1, space="PSUM"))

    # ---------------- persistent SBUF state
    W = perm.tile([128, 27 * 128], bf16)
    nc.sync.dma_start(W[:], t['wpk'][:])

    def w(i):
        return W[:, i * 128:(i + 1) * 128]

    bias = perm.tile([128, 15], f32)
    nc.sync.dma_start(bias[:], t['bpk'][:])

    def bv(i):
        return bias[:, i:i + 1]

    xT = perm.tile([128, cfg.NLP], f32)
    nc.sync.dma_start(xT[:], t['xT0'][:])
    xTb = perm.tile([128, cfg.NLP], bf16)
    nc.vector.tensor_copy(xTb[:], xT[:])

    uT = perm.tile([128, B], f32)
    nc.sync.dma_start(uT[:], t['uT0'][:])
    uTb = perm.tile([128, B], bf16)
    nc.vector.tensor_copy(uTb[:], uT[:])

    bsum_acc = perm.tile([128, B], f32)
    aggT = perm.tile([128, cfg.NLP], f32)     # resident aggregation accumulator

    ident_f = perm.tile([128, 128], f32)
    make_identity(nc, ident_f[:])
    ident_b = perm.tile([128, 128], bf16)
    nc.vector.tensor_copy(ident_b[:], ident_f[:])

    # ---------------- init DRAM state
    nc.sync.dma_start(t['eTd'][0][:], t['eT0'][:])
    nc.sync.dma_start(t['Tlo'][:], t['Tlo0'][:])
    nc.sync.dma_start(t['Thi'][:], t['Thi0'][:])
    nc.sync.dma_start(t['Tloc'][:], t['Tloc0'][:])

    def gru(xiT, hTb, wb, bb, pool, h_f32, out_tag, FD):
        """GRU tail: xiT bf16 [128,FD] (input through W2 fold), hTb bf16 [128,FD].
        If h_f32 given: blend in f32 in-place there and return None; else return
        a bf16 tile. wb: base index of Wih2 r,z,g then Whh r,z,g. bb: bias base."""
        pr = ps_g.tile([128, FD], f32, tag="pr")
        nc.tensor.matmul(pr[:], lhsT=w(wb + 0), rhs=xiT, start=True, stop=False)
        nc.tensor.matmul(pr[:], lhsT=w(wb + 3), rhs=hTb, start=False, stop=True)
        pz = ps_g.tile([128, FD], f32, tag="pz")
        nc.tensor.matmul(pz[:], lhsT=w(wb + 1), rhs=xiT, start=True, stop=False)
        nc.tensor.matmul(pz[:], lhsT=w(wb + 4), rhs=hTb, start=False, stop=True)
        pig = ps_g.tile([128, FD], f32, tag="pig")
        nc.tensor.matmul(pig[:], lhsT=w(wb + 2), rhs=xiT, start=True, stop=True)
        phg = ps_g.tile([128, FD], f32, tag="phg")
        nc.tensor.matmul(phg[:], lhsT=w(wb + 5), rhs=hTb, start=True, stop=True)

        r = pool.tile([128, FD], bf16, tag="r")
        nc.scalar.activation(r[:], pr[:], AF.Sigmoid, bias=bv(bb + 0))
        z = pool.tile([128, FD], bf16, tag="z")
        nc.scalar.activation(z[:], pz[:], AF.Sigmoid, bias=bv(bb + 1))
        hg = pool.tile([128, FD], bf16, tag="hg")
        nc.scalar.activation(hg[:], phg[:], AF.Identity, bias=bv(bb + 2))
        tm = pool.tile([128, FD], f32, tag="tm")
        nc.vector.tensor_tensor(tm[:], r[:], hg[:], op=ALU.mult)
        sp = pool.tile([128, FD], f32, tag="sp")
        nc.vector.tensor_tensor(sp[:], tm[:], pig[:], op=ALU.add)
        n = pool.tile([128, FD], bf16, tag="n")
        nc.scalar.activation(n[:], sp[:], AF.Tanh, bias=bv(bb + 3))

        hold = h_f32 if h_f32 is not None else hTb
        d = pool.tile([128, FD], f32, tag="d")
        nc.vector.tensor_tensor(d[:], hold, n[:], op=ALU.subtract)
        m = pool.tile([128, FD], f32, tag="m")
        nc.vector.tensor_tensor(m[:], z[:], d[:], op=ALU.mult)
        if h_f32 is not None:
            nc.vector.tensor_tensor(h_f32, n[:], m[:], op=ALU.add)
            return None
        hN = pool.tile([128, FD], bf16, tag=out_tag)
        nc.vector.tensor_tensor(hN[:], n[:], m[:], op=ALU.add)
        return hN

    # SWDGE queue assignment: Tile round-robins DMASW sems (8) over SWDGE
    # instructions in emission order; queue = ctr % num_queues keeps each sem
    # pinned to one queue (sem s -> queue s % 4).
    _swdge_ctr = [0]

    def self_qn(_):
        q = _swdge_ctr[0] % nc.num_swdge_queues
        _swdge_ctr[0] += 1
        return q

    aw_ps = None
    for s in range(cfg.STEPS):
        eT_r, eT_w = t['eTd'][s % 2], t['eTd'][(s + 1) % 2]
        nc.vector.memset(aggT[:], 0.0)

        # per-step u projections: uWd_row = u @ W1d.T ; uWnc_row = u @ Wn1c.T
        uprj = []
        for wi, tg in ((3, "uprj_e"), (12, "uprj_n")):
            p = ps_g.tile([B, 128], f32, tag="pr")
            nc.tensor.matmul(p[:], lhsT=uTb[:], rhs=w(wi), start=True, stop=True)
            srow = sb2.tile([B, 128], bf16, tag=tg)
            nc.vector.tensor_copy(srow[:], p[:])
            uprj.append(srow)
        uWd_row, uWnc_row = uprj

        # ================= EDGE PHASE =================
        GB = min(2048, cfg.EPAD)          # gather batch (edges per dma_gather)
        CPB = GB // CH
        g_lo_b = g_hi_b = g_dst_b = None
        for k in range(cfg.NCHE):
            ce = slice(k * CH, (k + 1) * CH)

            if k % CPB == 0:
                cb = slice((k * CH) // 16, (k * CH + GB) // 16)
                ilo = sb.tile([128, GB // 16], i16, tag="ilo")
                nc.sync.dma_start(ilo[:], t['gsrc_lo'][:, cb])
                ihi = sb.tile([128, GB // 16], i16, tag="ihi")
                nc.sync.dma_start(ihi[:], t['gsrc_hi'][:, cb])
                idl = sb.tile([128, GB // 16], i16, tag="idl")
                nc.sync.dma_start(idl[:], t['gdst'][:, cb])

                g_lo_b = sb.tile([128, 1, GB], bf16, tag="g_lo")
                nc.gpsimd.dma_gather(g_lo_b[:], t['Tlo'][:], ilo[:], GB, GB, H,
                                     transpose=True, single_packet=False,
                                     queue_num=self_qn(0))
                g_hi_b = sb.tile([128, 1, GB], bf16, tag="g_hi")
                nc.gpsimd.dma_gather(g_hi_b[:], t['Thi'][:], ihi[:], GB, GB, H,
                                     transpose=True, single_packet=False,
                                     queue_num=self_qn(1))
                g_dst_b = sb.tile([128, 1, GB], bf16, tag="g_dst")
                nc.gpsimd.dma_gather(g_dst_b[:], t['Tloc'][:], idl[:], GB, GB, H,
                                     transpose=True, single_packet=False,
                                     queue_num=self_qn(2))

            kk = (k % CPB) * CH
            g_lo = g_lo_b[:, 0, kk:kk + CH]
            g_hi = g_hi_b[:, 0, kk:kk + CH]
            g_dst = g_dst_b[:, 0, kk:kk + CH]

            eT_c = sb.tile([128, CH], bf16, tag="eT_c")
            nc.sync.dma_start(eT_c[:], eT_r[:, ce])
            su_c = sb.tile([B, CH], bf16, tag="su_c")
            nc.sync.dma_start(su_c[:], t['S_u'][:, ce])

            h1 = ps_h1.tile([128, CH], f32, tag="h1")
            nc.tensor.matmul(h1[:], lhsT=w(0), rhs=g_lo, start=True, stop=False)
            nc.tensor.matmul(h1[:], lhsT=w(0), rhs=g_hi, start=False, stop=False)
            nc.tensor.matmul(h1[:], lhsT=w(1), rhs=g_dst, start=False, stop=False)
            nc.tensor.matmul(h1[:], lhsT=w(2), rhs=eT_c[:], start=False, stop=False)
            nc.tensor.matmul(h1[:], lhsT=uWd_row[:], rhs=su_c[:], start=False, stop=True)

            rh1 = sb.tile([128, CH], bf16, tag="rh1")
            nc.scalar.activation(rh1[:], h1[:], AF.Relu, bias=bv(0))

            hN = gru(rh1[:], eT_c[:], 4, 1, sb, None, "hN", CH)
            nc.sync.dma_start(eT_w[:, ce], hN[:])

            # transpose to row-form, then aggregate via one-hot A matmuls
            tpp = ps_tp.tile([128, CH], bf16, tag="tp_b")
            for j in range(NSUB):
                nc.tensor.transpose(tpp[:, j * 128:(j + 1) * 128],
                                    hN[:, j * 128:(j + 1) * 128], ident_b[:])
            erow = sb.tile([128, CH], bf16, tag="erow")
            nc.vector.tensor_copy(erow[:], tpp[:])
            for j in range(NSUB):
                gs = k * NSUB + j
                wb = cfg.wstart[gs]
                first = (gs == 0) or (cfg.wstart[gs - 1] != wb)
                last = (gs == cfg.NSUBS - 1) or (cfg.wstart[gs + 1] != wb)
                atile = sb.tile([128, cfg.AW], bf16, tag="atile")
                nc.sync.dma_start(atile[:], t['Amat'][gs, :, :])
                if first:
                    aw_ps = ps_tp.tile([128, cfg.AW], f32, tag="aw")
                nc.tensor.matmul(aw_ps[:], lhsT=erow[:, j * 128:(j + 1) * 128],
                                 rhs=atile[:], start=first, stop=last)
                if last:
                    nc.vector.tensor_tensor(aggT[:, wb:wb + cfg.AW],
                                            aggT[:, wb:wb + cfg.AW],
                                            aw_ps[:], op=ALU.add)

        # ================= NODE PHASE =================
        for k in range(cfg.NCHN):
            cn = slice(k * CH, (k + 1) * CH)

            aggTb = sb.tile([128, CH], bf16, tag="aggTb")
            nc.vector.tensor_copy(aggTb[:], aggT[:, cn])

            snb_c = sb.tile([B, CH], bf16, tag="su_c")
            nc.sync.dma_start(snb_c[:], t['S_nb'][:, cn])

            h1 = ps_h1.tile([128, CH], f32, tag="h1")
            nc.tensor.matmul(h1[:], lhsT=w(10), rhs=xTb[:, cn], start=True, stop=False)
            nc.tensor.matmul(h1[:], lhsT=w(11), rhs=aggTb[:], start=False, stop=False)
            nc.tensor.matmul(h1[:], lhsT=uWnc_row[:], rhs=snb_c[:], start=False, stop=True)

            rh1 = sb.tile([128, CH], bf16, tag="rh1")
            nc.scalar.activation(rh1[:], h1[:], AF.Relu, bias=bv(5))

            gru(rh1[:], xTb[:, cn], 13, 6, sb, xT[:, cn], None, CH)
            nc.vector.tensor_copy(xTb[:, cn], xT[:, cn])

            # row-form x for AllGather input, local gather table, graph means
            bmat_c = sb.tile([128, NSUB, B], bf16, tag="bmat_c")
            for j in range(NSUB):
                nc.sync.dma_start(bmat_c[:, j, :],
                                  t['Bmat'][k * CH + j * 128: k * CH + (j + 1) * 128, :])
            bmm = ps_g.tile([128, B], f32, tag="pr")
            for j in range(NSUB):
                xtp = ps_tp.tile([128, 128], bf16, tag="tp_b")
                nc.tensor.transpose(xtp[:], xTb[:, k * CH + j * 128: k * CH + (j + 1) * 128],
                                    ident_b[:])
                xrow = sb.tile([128, 128], bf16, tag="xrow")
                nc.vector.tensor_copy(xrow[:], xtp[:])
                base = k * CH + j * 128
                nrows = max(0, min(128, cfg.NL - base))
                if nrows > 0 and s < cfg.STEPS - 1:
                    nc.sync.dma_start(t['x_shard'][base:base + nrows, :], xrow[:nrows, :])
                    nc.sync.dma_start(t['Tloc'][base:base + nrows, :], xrow[:nrows, :])
                nc.tensor.matmul(bmm[:], lhsT=xrow[:], rhs=bmat_c[:, j, :],
                                 start=(j == 0), stop=(j == NSUB - 1))
            if k == 0:
                nc.vector.tensor_copy(bsum_acc[:], bmm[:])
            else:
                nc.vector.tensor_tensor(bsum_acc[:], bsum_acc[:], bmm[:], op=ALU.add)

        # ================= GLOBAL PHASE =================
        nc.sync.dma_start(t['gsum_in'][:], bsum_acc[:])
        nc.gpsimd.collective_compute(
            "AllReduce", ALU.add, replica_groups=t['rg'],
            ins=[t['gsum_in'][:]], outs=[t['gsum_out'][:]])
        nmF = sb2.tile([128, B], f32, tag="nmF")
        nc.sync.dma_start(nmF[:], t['gsum_out'][:])
        nmT = sb2.tile([128, B], bf16, tag="nmT")
        nc.vector.tensor_copy(nmT[:], nmF[:])

        h1g = ps_h1.tile([128, B], f32, tag="h1")
        nc.tensor.matmul(h1g[:], lhsT=w(19), rhs=uTb[:], start=True, stop=False)
        nc.tensor.matmul(h1g[:], lhsT=w(20), rhs=nmT[:], start=False, stop=True)
        rh1g = sb2.tile([128, B], bf16, tag="rh1g")
        nc.scalar.activation(rh1g[:], h1g[:], AF.Relu, bias=bv(10))

        gru(rh1g[:], uTb[:], 21, 11, sb2, uT[:], None, B)
        nc.vector.tensor_copy(uTb[:], uT[:])

        utp = ps_tp.tile([B, 128], f32, tag="aw")
        nc.tensor.transpose(utp[:], uT[:], ident_f[:])
        urow = sb2.tile([B, 128], f32, tag="urow")
        nc.vector.tensor_copy(urow[:], utp[:])
        nc.sync.dma_start(t['out'][:, s, :], urow[:])

        # ================= AllGather x, rebuild tables =================
        if s < cfg.STEPS - 1:
            nc.gpsimd.collective_compute(
                "AllGather", ALU.bypass, replica_groups=t['rg'],
                ins=[t['x_shard'][:]], outs=[t['x_full'][:]])
            nc.sync.dma_start(t['Tlo'][0:cfg.LO_REAL, :], t['x_full'][0:cfg.LO_REAL, :])
            if cfg.HI_REAL > 0:
                nc.sync.dma_start(t['Thi'][0:cfg.HI_REAL, :],
                                  t['x_full'][cfg.LO_REAL:cfg.N, :])


# ---------------------------------------------------------------- entry point

_CACHE = {}


def kernel(**inputs):
    x = np.asarray(inputs['x'])
    ei = np.asarray(inputs['edge_index'])
    u = np.asarray(inputs['u'])
    cfg = Cfg(N=x.shape[0], E=ei.shape[1], B=u.shape[0], H=x.shape[1], STEPS=3)
    in_maps = host_prepare(cfg, inputs)
    key = (cfg.N, cfg.E, cfg.B, cfg.H, cfg.STEPS, cfg.EPAD)
    if key not in _CACHE:
        _CACHE[key] = build_program(cfg)
    nc = _CACHE[key]
    res = run_bass_kernel_spmd(nc, in_maps, list(range(cfg.NCORES)))
    return np.asarray(res.results[0]["out"], np.float32)


# revision 45
# speedup vs baseline: 1.8543x; 1.4082x over previous
"""Trainium2 Bass kernel for MetaLayer-style GNN (edge/node/global GRU message passing).

Contract: kernel(**inputs) takes the FULL unsharded inputs (np arrays, keys as in
setup_inputs) and returns the FULL output [B, STEPS, H] float32.

Strategy (8 NeuronCores):
- Sort edges by dst, shard nodes into 8 equal contiguous ranges; each core owns all
  edges whose dst is in its range => node aggregation is core-local.
- Per step: edge MLP+GRU (edge-parallel, bf16 matmuls, T-form activations),
  local segment-sum via PE-transpose + dma_scatter_add (fp32, DMA CCE adds),
  node MLP+GRU on local nodes, AllGather of updated x (bf16) to rebuild the
  replicated gather tables, small AllReduce for per-graph node means, replicated
  global MLP+GRU on every core.
- x and u kept resident in fp32 SBUF; MLP second layer folded into GRU input
  weights: gi = relu_h1 @ (Wih@W2).T + (Wih@b2 + bih).
"""

from contextlib import ExitStack

import numpy as np
import ml_dtypes

import concourse.bass as bass
import concourse.bacc as bacc
import concourse.tile as tile
from concourse import mybir
from concourse.bass_utils import run_bass_kernel_spmd
from concourse.masks import make_identity

BF16 = ml_dtypes.bfloat16
AF = mybir.ActivationFunctionType
DT = mybir.dt
ALU = mybir.AluOpType

# ---------------------------------------------------------------- configuration

class Cfg:
    def __init__(self, N=50000, E=500000, B=64, H=128, STEPS=3, NCORES=8,
                 CH=512, SCB=4096):
        assert H == 128
        assert N % NCORES == 0
        self.N, self.E, self.B, self.H, self.STEPS, self.NCORES = N, E, B, H, STEPS, NCORES
        self.CH = CH                      # edge chunk (free dim of f32 PSUM <= 512)
        self.SCB = SCB                    # edges per dma_scatter_add call
        self.NL = N // NCORES             # local nodes
        self.NLP = ((self.NL + CH - 1) // CH) * CH
        self.NCHN = self.NLP // CH        # node chunks
        self.LO_REAL = min(N, 32767)      # x rows in lo table (int16 index limit)
        self.HI_REAL = N - self.LO_REAL
        self.LO_ROWS = self.LO_REAL + 1   # + zero row
        self.HI_ROWS = self.HI_REAL + 1   # + zero row
        self.TLOC_ROWS = self.NL + 1      # + zero row

    def finalize(self, max_shard_edges):
        assert self.SCB % self.CH == 0
        self.EPAD = ((max_shard_edges + self.SCB - 1) // self.SCB) * self.SCB
        self.NCHE = self.EPAD // self.CH  # edge chunks
        self.NSUBS = self.EPAD // 128     # 128-edge subs (one A tile each)
        self.AW = 256                     # aggregation window width (nodes)
        # data-independent window start per sub (aligned 128, clamped)
        self.wstart = []
        for sub in range(self.NSUBS):
            c = (sub + 0.5) * 128 * self.NL / self.EPAD
            w = 128 * int(c // 128) - 64
            w = max(0, min(w, self.NLP - self.AW))
            self.wstart.append(w)
        # chunk-level windows for the x[dst] expansion matmuls (64-aligned)
        self.w2start = []
        for k in range(self.NCHE):
            c = (k + 0.5) * self.CH * self.NL / self.EPAD
            w = 64 * int((c - 96) // 64)
            w = max(0, min(w, self.NLP - self.AW))
            self.w2start.append(w)
        self.NBLK = self.NLP // 128       # PXrow blocks
        return self


# ---------------------------------------------------------------- host helpers

def _wrap16(idx, call):
    """Pack indices into the wrapped-16, replicated-128 layout of dma_gather /
    dma_scatter_add: element [p, c*(call//16) + s] = idx[c*call + s*16 + p%16]."""
    total = idx.shape[0]
    assert total % call == 0 and call % 16 == 0
    ncalls = total // call
    w = idx.reshape(ncalls, call // 16, 16)                   # [c, s, lane]
    w = np.transpose(w, (2, 0, 1)).reshape(16, total // 16)   # [lane, c*s]
    w = np.tile(w, (8, 1))                                    # -> 128 partitions
    return np.ascontiguousarray(w.astype(np.int16))


def _onehot(cols_idx, nrows, scale=None, dtype=BF16):
    """[nrows, len(cols_idx)]: out[cols_idx[j], j] = scale_j; idx<0 -> zero col."""
    ncols = cols_idx.shape[0]
    out = np.zeros((nrows, ncols), dtype=np.float32)
    j = np.nonzero(cols_idx >= 0)[0]
    s = np.ones(j.shape[0], np.float32) if scale is None else scale[j]
    out[cols_idx[j], j] = s
    return out.astype(dtype)


def host_prepare(cfg, inputs):
    N, E, B, H = cfg.N, cfg.E, cfg.B, cfg.H
    x = np.asarray(inputs['x'], np.float32)
    edge_index = np.asarray(inputs['edge_index'])
    edge_attr = np.asarray(inputs['edge_attr'], np.float32)
    u = np.asarray(inputs['u'], np.float32)
    batch = np.asarray(inputs['batch']).astype(np.int64)
    src, dst = edge_index[0].astype(np.int64), edge_index[1].astype(np.int64)

    def g(name):
        return np.asarray(inputs[name], np.float32)

    W1, b1 = g('edge_w1'), g('edge_b1')
    W2, b2 = g('edge_w2'), g('edge_b2')
    eWih, eWhh = g('egru_wih'), g('egru_whh')
    eBih, eBhh = g('egru_bih'), g('egru_bhh')
    nW1, nb1 = g('node_w1'), g('node_b1')
    nW2, nb2 = g('node_w2'), g('node_b2')
    nWih, nWhh = g('ngru_wih'), g('ngru_whh')
    nBih, nBhh = g('ngru_bih'), g('ngru_bhh')
    gW1, gb1 = g('glob_w1'), g('glob_b1')
    gW2, gb2 = g('glob_w2'), g('glob_b2')
    gWih, gWhh = g('ggru_wih'), g('ggru_whh')
    gBih, gBhh = g('ggru_bih'), g('ggru_bhh')

    eWih2, eBih2 = eWih @ W2, eWih @ b2 + eBih
    nWih2, nBih2 = nWih @ nW2, nWih @ nb2 + nBih
    gWih2, gBih2 = gWih @ gW2, gWih @ gb2 + gBih

    def gate(Wm, i):
        return Wm[i * H:(i + 1) * H, :].T

    blocks = [
        W1[:, 0:H].T, W1[:, H:2 * H].T, W1[:, 2 * H:3 * H].T, W1[:, 3 * H:4 * H].T,
        gate(eWih2, 0), gate(eWih2, 1), gate(eWih2, 2),
        gate(eWhh, 0), gate(eWhh, 1), gate(eWhh, 2),
        nW1[:, 0:H].T, nW1[:, H:2 * H].T, nW1[:, 2 * H:3 * H].T,
        gate(nWih2, 0), gate(nWih2, 1), gate(nWih2, 2),
        gate(nWhh, 0), gate(nWhh, 1), gate(nWhh, 2),
        gW1[:, 0:H].T, gW1[:, H:2 * H].T,
        gate(gWih2, 0), gate(gWih2, 1), gate(gWih2, 2),
        gate(gWhh, 0), gate(gWhh, 1), gate(gWhh, 2),
    ]
    wpk = np.concatenate([bl.astype(np.float32) for bl in blocks], axis=1).astype(BF16)

    def gb_(v, i):
        return v[i * H:(i + 1) * H]

    bcols = [
        b1, gb_(eBih2, 0) + gb_(eBhh, 0), gb_(eBih2, 1) + gb_(eBhh, 1), gb_(eBhh, 2), gb_(eBih2, 2),
        nb1, gb_(nBih2, 0) + gb_(nBhh, 0), gb_(nBih2, 1) + gb_(nBhh, 1), gb_(nBhh, 2), gb_(nBih2, 2),
        gb1, gb_(gBih2, 0) + gb_(gBhh, 0), gb_(gBih2, 1) + gb_(gBhh, 1), gb_(gBhh, 2), gb_(gBih2, 2),
    ]
    bpk = np.stack(bcols, axis=1).astype(np.float32)

    order = np.argsort(dst, kind='stable')
    ssrc, sdst, sea = src[order], dst[order], edge_attr[order]
    shard_of = sdst // cfg.NL
    counts = np.bincount(shard_of, minlength=cfg.NCORES)
    cfg.finalize(int(counts.max()))

    gcnt = np.bincount(batch, minlength=B).astype(np.float32)
    ginv = 1.0 / np.maximum(gcnt, 1.0)
    ncnt = np.bincount(sdst, minlength=N).astype(np.float32)
    ninv = 1.0 / np.maximum(ncnt, 1.0)
    bsrc_all = batch[ssrc]

    xb = x.astype(BF16)
    in_maps = []
    bounds = np.searchsorted(sdst, np.arange(cfg.NCORES + 1) * cfg.NL)
    for c in range(cfg.NCORES):
        lo_, hi_ = int(bounds[c]), int(bounds[c + 1])
        ne = hi_ - lo_
        npad = cfg.EPAD - ne
        base = c * cfg.NL
        nl, nlp = cfg.NL, cfg.NLP

        # Interleave pads uniformly so slot->node quantile mapping matches the
        # program-uniform window schedule (all-at-end padding would drift).
        pad_slots = np.unique(np.round(np.linspace(0, cfg.EPAD - 1, npad)).astype(np.int64)) \
            if npad > 0 else np.empty(0, np.int64)
        # numerical safety: ensure exactly npad distinct slots
        while pad_slots.shape[0] < npad:
            extra = np.setdiff1d(np.arange(cfg.EPAD), pad_slots)[:npad - pad_slots.shape[0]]
            pad_slots = np.union1d(pad_slots, extra)
        is_pad = np.zeros(cfg.EPAD, bool)
        is_pad[pad_slots] = True
        slot_edge = np.full(cfg.EPAD, -1, np.int64)
        slot_edge[~is_pad] = np.arange(ne)

        def scatter_edges(vals, padval):
            out = np.full(cfg.EPAD, padval, vals.dtype)
            out[~is_pad] = vals
            return out

        csrc = ssrc[lo_:hi_]
        cdst_loc = sdst[lo_:hi_] - base
        cbsrc = bsrc_all[lo_:hi_]

        eslot = np.nonzero(~is_pad)[0]                     # slot of real edge i

        # src pair-gather: idx = src//2 into x viewed as [N/2, 2H]; merge parity
        gpair = scatter_edges(csrc // 2, np.int64(0))
        pmask = np.zeros(cfg.EPAD, np.float32)
        pmask[eslot] = (csrc % 2).astype(np.float32)
        pmaskT = np.ascontiguousarray(
            np.broadcast_to(pmask[None, :], (128, cfg.EPAD))).astype(np.uint8)

        # D tiles: per chunk, expansion one-hot [2, 128, CH] mapping window
        # nodes -> edge columns (x[dst] = PXrow_window contraction).
        w2 = np.asarray(cfg.w2start)                       # [NCHE]
        rel2 = cdst_loc - w2[eslot // cfg.CH]
        assert rel2.min() >= 0 and rel2.max() < cfg.AW, \
            f"dst window violated: {rel2.min()} {rel2.max()}"
        Dmat = np.zeros((cfg.NCHE, 2, 128, cfg.CH), np.float32)
        Dmat[eslot // cfg.CH, rel2 // 128, rel2 % 128, eslot % cfg.CH] = 1.0
        Dmat = Dmat.astype(BF16)

        # A tiles: per 128-edge sub, one-hot [128, AW] with 1/cnt folded,
        # targeting the sub's data-independent window.
        ws = np.asarray(cfg.wstart)                        # [NSUBS]
        rel = cdst_loc - ws[eslot // 128]
        assert rel.min() >= 0 and rel.max() < cfg.AW, \
            f"agg window violated: {rel.min()} {rel.max()}"
        Amat = np.zeros((cfg.NSUBS, 128, cfg.AW), np.float32)
        ninv_loc = ninv[base:base + nl]
        Amat[eslot // 128, eslot % 128, rel] = ninv_loc[cdst_loc]
        Amat = Amat.astype(BF16)

        batch_loc = batch[base:base + nl]
        bl_pad = np.concatenate([batch_loc, np.full(nlp - nl, -1, np.int64)])

        xT0 = np.zeros((128, nlp), np.float32)
        xT0[:, :nl] = x[base:base + nl].T
        eT0 = np.zeros((128, cfg.EPAD), BF16)
        eT0[:, eslot] = sea[lo_:hi_].T.astype(BF16)

        in_maps.append(dict(
            wpk=wpk, bpk=bpk,
            xT0=xT0,
            uT0=np.ascontiguousarray(u.T).astype(np.float32),
            eT0=eT0,
            x0b=xb,
            gpair=_wrap16(gpair, min(2048, cfg.EPAD)),
            pmaskT=pmaskT,
            Dmat=Dmat,
            Amat=Amat,
            S_u=_onehot(scatter_edges(cbsrc, np.int64(-1)), B),
            S_nb=_onehot(bl_pad, B),
            Bmat=np.ascontiguousarray(
                _onehot(bl_pad, B, scale=ginv[np.clip(bl_pad, 0, B - 1)]).T),
        ))
    return in_maps


# ---------------------------------------------------------------- device program

def build_program(cfg):
    nc = bacc.Bacc("TRN2", target_bir_lowering=False, debug=False,
                   num_devices=cfg.NCORES, num_swdge_queues=4)
    H, B, CH = cfg.H, cfg.B, cfg.CH
    NW = 27
    f32, bf16, i16 = DT.float32, DT.bfloat16, DT.int16

    def din(name, shape, dt):
        return nc.dram_tensor(name, shape, dt, kind="ExternalInput").ap()

    t = {}
    t['wpk'] = din("wpk", [128, NW * 128], bf16)
    t['bpk'] = din("bpk", [128, 15], f32)
    t['xT0'] = din("xT0", [128, cfg.NLP], f32)
    t['uT0'] = din("uT0", [128, B], f32)
    t['eT0'] = din("eT0", [128, cfg.EPAD], bf16)
    t['x0b'] = din("x0b", [cfg.N, H], bf16)
    t['gpair'] = din("gpair", [128, cfg.EPAD // 16], i16)
    t['pmaskT'] = din("pmaskT", [128, cfg.EPAD], DT.uint8)
    t['Dmat'] = din("Dmat", [cfg.NCHE, 2, 128, CH], bf16)
    t['Amat'] = din("Amat", [cfg.NSUBS, 128, cfg.AW], bf16)
    t['S_u'] = din("S_u", [B, cfg.EPAD], bf16)
    t['S_nb'] = din("S_nb", [B, cfg.NLP], bf16)
    t['Bmat'] = din("Bmat", [cfg.NLP, B], bf16)

    t['out'] = nc.dram_tensor("out", [B, cfg.STEPS, H], f32, kind="ExternalOutput").ap()

    t['eTd'] = [nc.dram_tensor(f"eTd{i}", [128, cfg.EPAD], bf16).ap() for i in range(2)]
    t['x_shard'] = nc.dram_tensor("x_shard", [cfg.NL, H], bf16).ap()
    t['x_full'] = nc.dram_tensor("x_full", [cfg.N, H], bf16, addr_space="Shared").ap()
    t['gsum_in'] = nc.dram_tensor("gsum_in", [128, B], f32).ap()
    t['gsum_out'] = nc.dram_tensor("gsum_out", [128, B], f32, addr_space="Shared").ap()
    t['rg'] = [list(range(cfg.NCORES))]

    with ExitStack() as ctx:
        tc = ctx.enter_context(tile.TileContext(nc))
        _emit(nc, tc, ctx, cfg, t)
    nc.compile()
    return nc


def _emit(nc, tc, ctx, cfg, t):
    H, B, CH = cfg.H, cfg.B, cfg.CH
    f32, bf16, i16 = DT.float32, DT.bfloat16, DT.int16
    NSUB = CH // 128

    perm = ctx.enter_context(tc.tile_pool(name="perm", bufs=1))
    sb = ctx.enter_context(tc.tile_pool(name="sb", bufs=2))
    sb2 = ctx.enter_context(tc.tile_pool(name="sb2", bufs=2))
    ps_h1 = ctx.enter_context(tc.tile_pool(name="ps_h1", bufs=2, space="PSUM"))
    ps_g = ctx.enter_context(tc.tile_pool(name="ps_g", bufs=1, space="PSUM"))
    ps_tp = ctx.enter_context(tc.tile_pool(name="ps_tp", bufs=1, space="PSUM"))

    # ---------------- persistent SBUF state
    W = perm.tile([128, 27 * 128], bf16)
    nc.sync.dma_start(W[:], t['wpk'][:])

    def w(i):
        return W[:, i * 128:(i + 1) * 128]

    bias = perm.tile([128, 15], f32)
    nc.sync.dma_start(bias[:], t['bpk'][:])

    def bv(i):
        return bias[:, i:i + 1]

    xT = perm.tile([128, cfg.NLP], f32)
    nc.sync.dma_start(xT[:], t['xT0'][:])
    xTb = perm.tile([128, cfg.NLP], bf16)
    nc.vector.tensor_copy(xTb[:], xT[:])

    uT = perm.tile([128, B], f32)
    nc.sync.dma_start(uT[:], t['uT0'][:])
    uTb = perm.tile([128, B], bf16)
    nc.vector.tensor_copy(uTb[:], uT[:])

    bsum_acc = perm.tile([128, B], f32)
    aggT = perm.tile([128, cfg.NLP], f32)     # resident aggregation accumulator
    # W1b-projected x rows at two 64-node alignments (for the x[dst] expansion)
    PXa = perm.tile([128, cfg.NBLK, 128], bf16)
    PXb = perm.tile([128, cfg.NBLK, 128], bf16)

    ident_f = perm.tile([128, 128], f32)
    make_identity(nc, ident_f[:])
    ident_b = perm.tile([128, 128], bf16)
    nc.vector.tensor_copy(ident_b[:], ident_f[:])

    # ---------------- init DRAM state
    nc.sync.dma_start(t['eTd'][0][:], t['eT0'][:])
    nc.sync.dma_start(t['x_full'][:], t['x0b'][:])
    x_pair = t['x_full'].rearrange("(a two) h -> a (two h)", two=2)  # [N/2, 2H]

    def gru(xiT, hTb, wb, bb, pool, h_f32, out_tag, FD):
        """GRU tail: xiT bf16 [128,FD] (input through W2 fold), hTb bf16 [128,FD].
        If h_f32 given: blend in f32 in-place there and return None; else return
        a bf16 tile. wb: base index of Wih2 r,z,g then Whh r,z,g. bb: bias base."""
        pr = ps_g.tile([128, FD], f32, tag="pr")
        nc.tensor.matmul(pr[:], lhsT=w(wb + 0), rhs=xiT, start=True, stop=False)
        nc.tensor.matmul(pr[:], lhsT=w(wb + 3), rhs=hTb, start=False, stop=True)
        pz = ps_g.tile([128, FD], f32, tag="pz")
        nc.tensor.matmul(pz[:], lhsT=w(wb + 1), rhs=xiT, start=True, stop=False)
        nc.tensor.matmul(pz[:], lhsT=w(wb + 4), rhs=hTb, start=False, stop=True)
        pig = ps_g.tile([128, FD], f32, tag="pig")
        nc.tensor.matmul(pig[:], lhsT=w(wb + 2), rhs=xiT, start=True, stop=True)
        phg = ps_g.tile([128, FD], f32, tag="phg")
        nc.tensor.matmul(phg[:], lhsT=w(wb + 5), rhs=hTb, start=True, stop=True)

        r = pool.tile([128, FD], bf16, tag="r")
        nc.scalar.activation(r[:], pr[:], AF.Sigmoid, bias=bv(bb + 0))
        z = pool.tile([128, FD], bf16, tag="z")
        nc.scalar.activation(z[:], pz[:], AF.Sigmoid, bias=bv(bb + 1))
        hg = pool.tile([128, FD], bf16, tag="hg")
        nc.scalar.activation(hg[:], phg[:], AF.Identity, bias=bv(bb + 2))
        tm = pool.tile([128, FD], f32, tag="tm")
        nc.vector.tensor_tensor(tm[:], r[:], hg[:], op=ALU.mult)
        sp = pool.tile([128, FD], f32, tag="sp")
        nc.vector.tensor_tensor(sp[:], tm[:], pig[:], op=ALU.add)
        n = pool.tile([128, FD], bf16, tag="n")
        nc.scalar.activation(n[:], sp[:], AF.Tanh, bias=bv(bb + 3))

        hold = h_f32 if h_f32 is not None else hTb
        d = pool.tile([128, FD], f32, tag="d")
        nc.vector.tensor_tensor(d[:], hold, n[:], op=ALU.subtract)
        m = pool.tile([128, FD], f32, tag="m")
        nc.vector.tensor_tensor(m[:], z[:], d[:], op=ALU.mult)
        if h_f32 is not None:
            nc.vector.tensor_tensor(h_f32, n[:], m[:], op=ALU.add)
            return None
        hN = pool.tile([128, FD], bf16, tag=out_tag)
        nc.vector.tensor_tensor(hN[:], n[:], m[:], op=ALU.add)
        return hN

    # SWDGE queue assignment: Tile round-robins DMASW sems (8) over SWDGE
    # instructions in emission order; queue = ctr % num_queues keeps each sem
    # pinned to one queue (sem s -> queue s % 4).
    _swdge_ctr = [0]

    def self_qn(_):
        q = _swdge_ctr[0] % nc.num_swdge_queues
        _swdge_ctr[0] += 1
        return q

    aw_ps = None
    for s in range(cfg.STEPS):
        eT_r, eT_w = t['eTd'][s % 2], t['eTd'][(s + 1) % 2]
        nc.vector.memset(aggT[:], 0.0)

        # per-step u projections: uWd_row = u @ W1d.T ; uWnc_row = u @ Wn1c.T
        uprj = []
        for wi, tg in ((3, "uprj_e"), (12, "uprj_n")):
            p = ps_g.tile([B, 128], f32, tag="pr")
            nc.tensor.matmul(p[:], lhsT=uTb[:], rhs=w(wi), start=True, stop=True)
            srow = sb2.tile([B, 128], bf16, tag=tg)
            nc.vector.tensor_copy(srow[:], p[:])
            uprj.append(srow)
        uWd_row, uWnc_row = uprj

        # PXrow: per 128-node block, rows of x @ W1b.T (two 64-node alignments)
        for dstn, off in ((PXa, 0), (PXb, 64)):
            for blk in range(cfg.NBLK):
                base = off + blk * 128
                wid = min(128, cfg.NLP - base)
                if wid <= 0:
                    break
                px = ps_h1.tile([128, 128], f32, tag="h1")
                nc.tensor.matmul(px[:wid, :], lhsT=xTb[:, base:base + wid],
                                 rhs=w(1), start=True, stop=True)
                nc.vector.tensor_copy(dstn[:, blk, :][:wid, :], px[:wid, :])

        # ================= EDGE PHASE =================
        GB = min(2048, cfg.EPAD)          # gather batch (edges per dma_gather)
        CPB = GB // CH
        g_pair_b = None
        for k in range(cfg.NCHE):
            ce = slice(k * CH, (k + 1) * CH)

            if k % CPB == 0:
                cb = slice((k * CH) // 16, (k * CH + GB) // 16)
                ipr = sb.tile([128, GB // 16], i16, tag="ipr")
                nc.sync.dma_start(ipr[:], t['gpair'][:, cb])
                g_pair_b = sb.tile([128, 2, GB], bf16, tag="g_pair")
                nc.gpsimd.dma_gather(g_pair_b[:], x_pair, ipr[:], GB, GB, 2 * H,
                                     transpose=True, single_packet=False,
                                     queue_num=self_qn(0))

            kk = (k % CPB) * CH
            # parity merge in place: even slot := odd where src odd
            pm = sb.tile([128, CH], DT.uint8, tag="pm")
            nc.sync.dma_start(pm[:], t['pmaskT'][:, ce])
            nc.vector.copy_predicated(g_pair_b[:, 0, kk:kk + CH], pm[:],
                                      g_pair_b[:, 1, kk:kk + CH])
            g_src = g_pair_b[:, 0, kk:kk + CH]

            eT_c = sb.tile([128, CH], bf16, tag="eT_c")
            nc.sync.dma_start(eT_c[:], eT_r[:, ce])
            su_c = sb.tile([B, CH], bf16, tag="su_c")
            nc.sync.dma_start(su_c[:], t['S_u'][:, ce])
            d0 = sb.tile([128, CH], bf16, tag="d0")
            nc.sync.dma_start(d0[:], t['Dmat'][k, 0, :, :])
            d1 = sb.tile([128, CH], bf16, tag="d1")
            nc.sync.dma_start(d1[:], t['Dmat'][k, 1, :, :])

            w2 = cfg.w2start[k]
            if w2 % 128 == 0:
                pxh0 = PXa[:, w2 // 128, :]
                pxh1 = PXa[:, w2 // 128 + 1, :]
            else:
                pxh0 = PXb[:, (w2 - 64) // 128, :]
                pxh1 = PXb[:, (w2 - 64) // 128 + 1, :]

            h1 = ps_h1.tile([128, CH], f32, tag="h1")
            nc.tensor.matmul(h1[:], lhsT=w(0), rhs=g_src, start=True, stop=False)
            nc.tensor.matmul(h1[:], lhsT=pxh0, rhs=d0[:], start=False, stop=False)
            nc.tensor.matmul(h1[:], lhsT=pxh1, rhs=d1[:], start=False, stop=False)
            nc.tensor.matmul(h1[:], lhsT=w(2), rhs=eT_c[:], start=False, stop=False)
            nc.tensor.matmul(h1[:], lhsT=uWd_row[:], rhs=su_c[:], start=False, stop=True)

            rh1 = sb.tile([128, CH], bf16, tag="rh1")
            nc.scalar.activation(rh1[:], h1[:], AF.Relu, bias=bv(0))

            hN = gru(rh1[:], eT_c[:], 4, 1, sb, None, "hN", CH)
            nc.sync.dma_start(eT_w[:, ce], hN[:])

            # transpose to row-form, then aggregate via one-hot A matmuls
            tpp = ps_tp.tile([128, CH], bf16, tag="tp_b")
            for j in range(NSUB):
                nc.tensor.transpose(tpp[:, j * 128:(j + 1) * 128],
                                    hN[:, j * 128:(j + 1) * 128], ident_b[:])
            erow = sb.tile([128, CH], bf16, tag="erow")
            nc.vector.tensor_copy(erow[:], tpp[:])
            for j in range(NSUB):
                gs = k * NSUB + j
                wb = cfg.wstart[gs]
                first = (gs == 0) or (cfg.wstart[gs - 1] != wb)
                last = (gs == cfg.NSUBS - 1) or (cfg.wstart[gs + 1] != wb)
                atile = sb.tile([128, cfg.AW], bf16, tag="atile")
                nc.sync.dma_start(atile[:], t['Amat'][gs, :, :])
                if first:
                    aw_ps = ps_tp.tile([128, cfg.AW], f32, tag="aw")
                nc.tensor.matmul(aw_ps[:], lhsT=erow[:, j * 128:(j + 1) * 128],
                                 rhs=atile[:], start=first, stop=last)
                if last:
                    nc.vector.tensor_tensor(aggT[:, wb:wb + cfg.AW],
                                            aggT[:, wb:wb + cfg.AW],
                                            aw_ps[:], op=ALU.add)

        # ================= NODE PHASE =================
        for k in range(cfg.NCHN):
            cn = slice(k * CH, (k + 1) * CH)

            aggTb = sb.tile([128, CH], bf16, tag="aggTb")
            nc.vector.tensor_copy(aggTb[:], aggT[:, cn])

            snb_c = sb.tile([B, CH], bf16, tag="su_c")
            nc.sync.dma_start(snb_c[:], t['S_nb'][:, cn])

            h1 = ps_h1.tile([128, CH], f32, tag="h1")
            nc.tensor.matmul(h1[:], lhsT=w(10), rhs=xTb[:, cn], start=True, stop=False)
            nc.tensor.matmul(h1[:], lhsT=w(11), rhs=aggTb[:], start=False, stop=False)
            nc.tensor.matmul(h1[:], lhsT=uWnc_row[:], rhs=snb_c[:], start=False, stop=True)

            rh1 = sb.tile([128, CH], bf16, tag="rh1")
            nc.scalar.activation(rh1[:], h1[:], AF.Relu, bias=bv(5))

            gru(rh1[:], xTb[:, cn], 13, 6, sb, xT[:, cn], None, CH)
            nc.vector.tensor_copy(xTb[:, cn], xT[:, cn])

            # row-form x for AllGather input, local gather table, graph means
            bmat_c = sb.tile([128, NSUB, B], bf16, tag="bmat_c")
            for j in range(NSUB):
                nc.sync.dma_start(bmat_c[:, j, :],
                                  t['Bmat'][k * CH + j * 128: k * CH + (j + 1) * 128, :])
            bmm = ps_g.tile([128, B], f32, tag="pr")
            for j in range(NSUB):
                xtp = ps_tp.tile([128, 128], bf16, tag="tp_b")
                nc.tensor.transpose(xtp[:], xTb[:, k * CH + j * 128: k * CH + (j + 1) * 128],
                                    ident_b[:])
                xrow = sb.tile([128, 128], bf16, tag="xrow")
                nc.vector.tensor_copy(xrow[:], xtp[:])
                base = k * CH + j * 128
                nrows = max(0, min(128, cfg.NL - base))
                if nrows > 0 and s < cfg.STEPS - 1:
                    nc.sync.dma_start(t['x_shard'][base:base + nrows, :], xrow[:nrows, :])
                nc.tensor.matmul(bmm[:], lhsT=xrow[:], rhs=bmat_c[:, j, :],
                                 start=(j == 0), stop=(j == NSUB - 1))
            if k == 0:
                nc.vector.tensor_copy(bsum_acc[:], bmm[:])
            else:
                nc.vector.tensor_tensor(bsum_acc[:], bsum_acc[:], bmm[:], op=ALU.add)

        # ================= GLOBAL PHASE =================
        nc.sync.dma_start(t['gsum_in'][:], bsum_acc[:])
        nc.gpsimd.collective_compute(
            "AllReduce", ALU.add, replica_groups=t['rg'],
            ins=[t['gsum_in'][:]], outs=[t['gsum_out'][:]])
        nmF = sb2.tile([128, B], f32, tag="nmF")
        nc.sync.dma_start(nmF[:], t['gsum_out'][:])
        nmT = sb2.tile([128, B], bf16, tag="nmT")
        nc.vector.tensor_copy(nmT[:], nmF[:])

        h1g = ps_h1.tile([128, B], f32, tag="h1")
        nc.tensor.matmul(h1g[:], lhsT=w(19), rhs=uTb[:], start=True, stop=False)
        nc.tensor.matmul(h1g[:], lhsT=w(20), rhs=nmT[:], start=False, stop=True)
        rh1g = sb2.tile([128, B], bf16, tag="rh1g")
        nc.scalar.activation(rh1g[:], h1g[:], AF.Relu, bias=bv(10))

        gru(rh1g[:], uTb[:], 21, 11, sb2, uT[:], None, B)
        nc.vector.tensor_copy(uTb[:], uT[:])

        utp = ps_tp.tile([B, 128], f32, tag="aw")
        nc.tensor.transpose(utp[:], uT[:], ident_f[:])
        urow = sb2.tile([B, 128], f32, tag="urow")
        nc.vector.tensor_copy(urow[:], utp[:])
        nc.sync.dma_start(t['out'][:, s, :], urow[:])

        # ================= AllGather x (x_full doubles as the gather table) ==
        if s < cfg.STEPS - 1:
            nc.gpsimd.collective_compute(
                "AllGather", ALU.bypass, replica_groups=t['rg'],
                ins=[t['x_shard'][:]], outs=[t['x_full'][:]])


# ---------------------------------------------------------------- entry point

_CACHE = {}


def kernel(**inputs):
    x = np.asarray(inputs['x'])
    ei = np.asarray(inputs['edge_index'])
    u = np.asarray(inputs['u'])
    cfg = Cfg(N=x.shape[0], E=ei.shape[1], B=u.shape[0], H=x.shape[1], STEPS=3)
    in_maps = host_prepare(cfg, inputs)
    key = (cfg.N, cfg.E, cfg.B, cfg.H, cfg.STEPS, cfg.EPAD)
    if key not in _CACHE:
        _CACHE[key] = build_program(cfg)
    nc = _CACHE[key]
    res = run_bass_kernel_spmd(nc, in_maps, list(range(cfg.NCORES)))
    return np.asarray(res.results[0]["out"], np.float32)


# revision 46
# speedup vs baseline: 1.9349x; 1.0435x over previous
"""Trainium2 Bass kernel for MetaLayer-style GNN (edge/node/global GRU message passing).

Contract: kernel(**inputs) takes the FULL unsharded inputs (np arrays, keys as in
setup_inputs) and returns the FULL output [B, STEPS, H] float32.

Strategy (8 NeuronCores):
- Sort edges by dst, shard nodes into 8 equal contiguous ranges; each core owns all
  edges whose dst is in its range => node aggregation is core-local.
- Per step: edge MLP+GRU (edge-parallel, bf16 matmuls, T-form activations),
  local segment-sum via PE-transpose + dma_scatter_add (fp32, DMA CCE adds),
  node MLP+GRU on local nodes, AllGather of updated x (bf16) to rebuild the
  replicated gather tables, small AllReduce for per-graph node means, replicated
  global MLP+GRU on every core.
- x and u kept resident in fp32 SBUF; MLP second layer folded into GRU input
  weights: gi = relu_h1 @ (Wih@W2).T + (Wih@b2 + bih).
"""

from contextlib import ExitStack

import numpy as np
import ml_dtypes

import concourse.bass as bass
import concourse.bacc as bacc
import concourse.tile as tile
from concourse import mybir
from concourse.bass_utils import run_bass_kernel_spmd
from concourse.masks import make_identity

BF16 = ml_dtypes.bfloat16
AF = mybir.ActivationFunctionType
DT = mybir.dt
ALU = mybir.AluOpType

# ---------------------------------------------------------------- configuration

class Cfg:
    def __init__(self, N=50000, E=500000, B=64, H=128, STEPS=3, NCORES=8,
                 CH=512, SCB=4096):
        assert H == 128
        assert N % NCORES == 0
        self.N, self.E, self.B, self.H, self.STEPS, self.NCORES = N, E, B, H, STEPS, NCORES
        self.CH = CH                      # edge chunk (free dim of f32 PSUM <= 512)
        self.SCB = SCB                    # edges per dma_scatter_add call
        self.NL = N // NCORES             # local nodes
        self.NLP = ((self.NL + CH - 1) // CH) * CH
        self.NCHN = self.NLP // CH        # node chunks
        self.LO_REAL = min(N, 32767)      # x rows in lo table (int16 index limit)
        self.HI_REAL = N - self.LO_REAL
        self.LO_ROWS = self.LO_REAL + 1   # + zero row
        self.HI_ROWS = self.HI_REAL + 1   # + zero row
        self.TLOC_ROWS = self.NL + 1      # + zero row

    def finalize(self, max_shard_edges):
        assert self.SCB % self.CH == 0
        self.EPAD = ((max_shard_edges + self.SCB - 1) // self.SCB) * self.SCB
        self.NCHE = self.EPAD // self.CH  # edge chunks
        self.NSUBS = self.EPAD // 128     # 128-edge subs (one A tile each)
        self.AW = 256                     # aggregation window width (nodes)
        # data-independent window start per sub (aligned 128, clamped)
        self.wstart = []
        for sub in range(self.NSUBS):
            c = (sub + 0.5) * 128 * self.NL / self.EPAD
            w = 128 * int(c // 128) - 64
            w = max(0, min(w, self.NLP - self.AW))
            self.wstart.append(w)
        # chunk-level windows for the x[dst] expansion matmuls (64-aligned)
        self.w2start = []
        for k in range(self.NCHE):
            c = (k + 0.5) * self.CH * self.NL / self.EPAD
            w = 64 * int((c - 96) // 64)
            w = max(0, min(w, self.NLP - self.AW))
            self.w2start.append(w)
        self.NBLK = self.NLP // 128       # PXrow blocks
        return self


# ---------------------------------------------------------------- host helpers

def _wrap16(idx, call):
    """Pack indices into the wrapped-16, replicated-128 layout of dma_gather /
    dma_scatter_add: element [p, c*(call//16) + s] = idx[c*call + s*16 + p%16]."""
    total = idx.shape[0]
    assert total % call == 0 and call % 16 == 0
    ncalls = total // call
    w = idx.reshape(ncalls, call // 16, 16)                   # [c, s, lane]
    w = np.transpose(w, (2, 0, 1)).reshape(16, total // 16)   # [lane, c*s]
    w = np.tile(w, (8, 1))                                    # -> 128 partitions
    return np.ascontiguousarray(w.astype(np.int16))


def _onehot(cols_idx, nrows, scale=None, dtype=BF16):
    """[nrows, len(cols_idx)]: out[cols_idx[j], j] = scale_j; idx<0 -> zero col."""
    ncols = cols_idx.shape[0]
    out = np.zeros((nrows, ncols), dtype=np.float32)
    j = np.nonzero(cols_idx >= 0)[0]
    s = np.ones(j.shape[0], np.float32) if scale is None else scale[j]
    out[cols_idx[j], j] = s
    return out.astype(dtype)


def host_prepare(cfg, inputs):
    N, E, B, H = cfg.N, cfg.E, cfg.B, cfg.H
    x = np.asarray(inputs['x'], np.float32)
    edge_index = np.asarray(inputs['edge_index'])
    edge_attr = np.asarray(inputs['edge_attr'], np.float32)
    u = np.asarray(inputs['u'], np.float32)
    batch = np.asarray(inputs['batch']).astype(np.int64)
    src, dst = edge_index[0].astype(np.int64), edge_index[1].astype(np.int64)

    def g(name):
        return np.asarray(inputs[name], np.float32)

    W1, b1 = g('edge_w1'), g('edge_b1')
    W2, b2 = g('edge_w2'), g('edge_b2')
    eWih, eWhh = g('egru_wih'), g('egru_whh')
    eBih, eBhh = g('egru_bih'), g('egru_bhh')
    nW1, nb1 = g('node_w1'), g('node_b1')
    nW2, nb2 = g('node_w2'), g('node_b2')
    nWih, nWhh = g('ngru_wih'), g('ngru_whh')
    nBih, nBhh = g('ngru_bih'), g('ngru_bhh')
    gW1, gb1 = g('glob_w1'), g('glob_b1')
    gW2, gb2 = g('glob_w2'), g('glob_b2')
    gWih, gWhh = g('ggru_wih'), g('ggru_whh')
    gBih, gBhh = g('ggru_bih'), g('ggru_bhh')

    eWih2, eBih2 = eWih @ W2, eWih @ b2 + eBih
    nWih2, nBih2 = nWih @ nW2, nWih @ nb2 + nBih
    gWih2, gBih2 = gWih @ gW2, gWih @ gb2 + gBih

    def gate(Wm, i):
        return Wm[i * H:(i + 1) * H, :].T

    blocks = [
        W1[:, 0:H].T, W1[:, H:2 * H].T, W1[:, 2 * H:3 * H].T, W1[:, 3 * H:4 * H].T,
        gate(eWih2, 0), gate(eWih2, 1), gate(eWih2, 2),
        gate(eWhh, 0), gate(eWhh, 1), gate(eWhh, 2),
        nW1[:, 0:H].T, nW1[:, H:2 * H].T, nW1[:, 2 * H:3 * H].T,
        gate(nWih2, 0), gate(nWih2, 1), gate(nWih2, 2),
        gate(nWhh, 0), gate(nWhh, 1), gate(nWhh, 2),
        gW1[:, 0:H].T, gW1[:, H:2 * H].T,
        gate(gWih2, 0), gate(gWih2, 1), gate(gWih2, 2),
        gate(gWhh, 0), gate(gWhh, 1), gate(gWhh, 2),
    ]
    wpk = np.concatenate([bl.astype(np.float32) for bl in blocks], axis=1).astype(BF16)

    def gb_(v, i):
        return v[i * H:(i + 1) * H]

    bcols = [
        b1, gb_(eBih2, 0) + gb_(eBhh, 0), gb_(eBih2, 1) + gb_(eBhh, 1), gb_(eBhh, 2), gb_(eBih2, 2),
        nb1, gb_(nBih2, 0) + gb_(nBhh, 0), gb_(nBih2, 1) + gb_(nBhh, 1), gb_(nBhh, 2), gb_(nBih2, 2),
        gb1, gb_(gBih2, 0) + gb_(gBhh, 0), gb_(gBih2, 1) + gb_(gBhh, 1), gb_(gBhh, 2), gb_(gBih2, 2),
    ]
    bpk = np.stack(bcols, axis=1).astype(np.float32)

    order = np.argsort(dst, kind='stable')
    ssrc, sdst, sea = src[order], dst[order], edge_attr[order]
    shard_of = sdst // cfg.NL
    counts = np.bincount(shard_of, minlength=cfg.NCORES)
    cfg.finalize(int(counts.max()))

    gcnt = np.bincount(batch, minlength=B).astype(np.float32)
    ginv = 1.0 / np.maximum(gcnt, 1.0)
    ncnt = np.bincount(sdst, minlength=N).astype(np.float32)
    ninv = 1.0 / np.maximum(ncnt, 1.0)
    bsrc_all = batch[ssrc]

    xb = x.astype(BF16)
    in_maps = []
    bounds = np.searchsorted(sdst, np.arange(cfg.NCORES + 1) * cfg.NL)
    for c in range(cfg.NCORES):
        lo_, hi_ = int(bounds[c]), int(bounds[c + 1])
        ne = hi_ - lo_
        npad = cfg.EPAD - ne
        base = c * cfg.NL
        nl, nlp = cfg.NL, cfg.NLP

        # Interleave pads uniformly so slot->node quantile mapping matches the
        # program-uniform window schedule (all-at-end padding would drift).
        pad_slots = np.unique(np.round(np.linspace(0, cfg.EPAD - 1, npad)).astype(np.int64)) \
            if npad > 0 else np.empty(0, np.int64)
        # numerical safety: ensure exactly npad distinct slots
        while pad_slots.shape[0] < npad:
            extra = np.setdiff1d(np.arange(cfg.EPAD), pad_slots)[:npad - pad_slots.shape[0]]
            pad_slots = np.union1d(pad_slots, extra)
        is_pad = np.zeros(cfg.EPAD, bool)
        is_pad[pad_slots] = True
        slot_edge = np.full(cfg.EPAD, -1, np.int64)
        slot_edge[~is_pad] = np.arange(ne)

        def scatter_edges(vals, padval):
            out = np.full(cfg.EPAD, padval, vals.dtype)
            out[~is_pad] = vals
            return out

        csrc = ssrc[lo_:hi_]
        cdst_loc = sdst[lo_:hi_] - base
        cbsrc = bsrc_all[lo_:hi_]

        eslot = np.nonzero(~is_pad)[0]                     # slot of real edge i

        # src pair-gather: idx = src//2 into x viewed as [N/2, 2H]; merge parity
        gpair = scatter_edges(csrc // 2, np.int64(0))
        pmask = np.zeros(cfg.EPAD, np.float32)
        pmask[eslot] = (csrc % 2).astype(np.float32)
        pmaskT = np.ascontiguousarray(
            np.broadcast_to(pmask[None, :], (128, cfg.EPAD))).astype(np.uint8)

        # D tiles: per chunk, expansion one-hot [2, 128, CH] mapping window
        # nodes -> edge columns (x[dst] = PXrow_window contraction).
        w2 = np.asarray(cfg.w2start)                       # [NCHE]
        rel2 = cdst_loc - w2[eslot // cfg.CH]
        assert rel2.min() >= 0 and rel2.max() < cfg.AW, \
            f"dst window violated: {rel2.min()} {rel2.max()}"
        Dmat = np.zeros((cfg.NCHE, 2, 128, cfg.CH), np.float32)
        Dmat[eslot // cfg.CH, rel2 // 128, rel2 % 128, eslot % cfg.CH] = 1.0
        Dmat = Dmat.astype(BF16)

        # A tiles: per 128-edge sub, one-hot [128, AW] with 1/cnt folded,
        # targeting the sub's data-independent window.
        ws = np.asarray(cfg.wstart)                        # [NSUBS]
        rel = cdst_loc - ws[eslot // 128]
        assert rel.min() >= 0 and rel.max() < cfg.AW, \
            f"agg window violated: {rel.min()} {rel.max()}"
        Amat = np.zeros((cfg.NSUBS, 128, cfg.AW), np.float32)
        ninv_loc = ninv[base:base + nl]
        Amat[eslot // 128, eslot % 128, rel] = ninv_loc[cdst_loc]
        Amat = Amat.astype(BF16)

        batch_loc = batch[base:base + nl]
        bl_pad = np.concatenate([batch_loc, np.full(nlp - nl, -1, np.int64)])

        xT0 = np.zeros((128, nlp), np.float32)
        xT0[:, :nl] = x[base:base + nl].T
        eT0 = np.zeros((128, cfg.EPAD), BF16)
        eT0[:, eslot] = sea[lo_:hi_].T.astype(BF16)

        in_maps.append(dict(
            wpk=wpk, bpk=bpk,
            xT0=xT0,
            uT0=np.ascontiguousarray(u.T).astype(np.float32),
            eT0=eT0,
            x0b=xb,
            gpair=_wrap16(gpair, min(2048, cfg.EPAD)),
            pmaskT=pmaskT,
            Dmat=Dmat,
            Amat=Amat,
            S_u=_onehot(scatter_edges(cbsrc, np.int64(-1)), B),
            S_nb=_onehot(bl_pad, B),
            Bmat=np.ascontiguousarray(
                _onehot(bl_pad, B, scale=ginv[np.clip(bl_pad, 0, B - 1)]).T),
        ))
    return in_maps


# ---------------------------------------------------------------- device program

def build_program(cfg):
    nc = bacc.Bacc("TRN2", target_bir_lowering=False, debug=False,
                   num_devices=cfg.NCORES, num_swdge_queues=4)
    H, B, CH = cfg.H, cfg.B, cfg.CH
    NW = 27
    f32, bf16, i16 = DT.float32, DT.bfloat16, DT.int16

    def din(name, shape, dt):
        return nc.dram_tensor(name, shape, dt, kind="ExternalInput").ap()

    t = {}
    t['wpk'] = din("wpk", [128, NW * 128], bf16)
    t['bpk'] = din("bpk", [128, 15], f32)
    t['xT0'] = din("xT0", [128, cfg.NLP], f32)
    t['uT0'] = din("uT0", [128, B], f32)
    t['eT0'] = din("eT0", [128, cfg.EPAD], bf16)
    t['x0b'] = din("x0b", [cfg.N, H], bf16)
    t['gpair'] = din("gpair", [128, cfg.EPAD // 16], i16)
    t['pmaskT'] = din("pmaskT", [128, cfg.EPAD], DT.uint8)
    t['Dmat'] = din("Dmat", [cfg.NCHE, 2, 128, CH], bf16)
    t['Amat'] = din("Amat", [cfg.NSUBS, 128, cfg.AW], bf16)
    t['S_u'] = din("S_u", [B, cfg.EPAD], bf16)
    t['S_nb'] = din("S_nb", [B, cfg.NLP], bf16)
    t['Bmat'] = din("Bmat", [cfg.NLP, B], bf16)

    t['out'] = nc.dram_tensor("out", [B, cfg.STEPS, H], f32, kind="ExternalOutput").ap()

    t['eTd'] = [nc.dram_tensor(f"eTd{i}", [128, cfg.EPAD], bf16).ap() for i in range(2)]
    t['x_shard'] = nc.dram_tensor("x_shard", [cfg.NL, H], bf16).ap()
    t['x_full'] = nc.dram_tensor("x_full", [cfg.N, H], bf16, addr_space="Shared").ap()
    t['gsum_in'] = nc.dram_tensor("gsum_in", [128, B], f32).ap()
    t['gsum_out'] = nc.dram_tensor("gsum_out", [128, B], f32, addr_space="Shared").ap()
    t['rg'] = [list(range(cfg.NCORES))]

    with ExitStack() as ctx:
        tc = ctx.enter_context(tile.TileContext(nc))
        _emit(nc, tc, ctx, cfg, t)
    nc.compile()
    return nc


def _emit(nc, tc, ctx, cfg, t):
    H, B, CH = cfg.H, cfg.B, cfg.CH
    f32, bf16, i16 = DT.float32, DT.bfloat16, DT.int16
    NSUB = CH // 128

    perm = ctx.enter_context(tc.tile_pool(name="perm", bufs=1))
    sb = ctx.enter_context(tc.tile_pool(name="sb", bufs=3))
    sb2 = ctx.enter_context(tc.tile_pool(name="sb2", bufs=2))
    ps_h1 = ctx.enter_context(tc.tile_pool(name="ps_h1", bufs=2, space="PSUM"))
    ps_g = ctx.enter_context(tc.tile_pool(name="ps_g", bufs=1, space="PSUM"))
    ps_tp = ctx.enter_context(tc.tile_pool(name="ps_tp", bufs=1, space="PSUM"))

    # ---------------- persistent SBUF state
    W = perm.tile([128, 27 * 128], bf16)
    nc.sync.dma_start(W[:], t['wpk'][:])

    def w(i):
        return W[:, i * 128:(i + 1) * 128]

    bias = perm.tile([128, 15], f32)
    nc.sync.dma_start(bias[:], t['bpk'][:])

    def bv(i):
        return bias[:, i:i + 1]

    xT = perm.tile([128, cfg.NLP], f32)
    nc.sync.dma_start(xT[:], t['xT0'][:])
    xTb = perm.tile([128, cfg.NLP], bf16)
    nc.vector.tensor_copy(xTb[:], xT[:])

    uT = perm.tile([128, B], f32)
    nc.sync.dma_start(uT[:], t['uT0'][:])
    uTb = perm.tile([128, B], bf16)
    nc.vector.tensor_copy(uTb[:], uT[:])

    bsum_acc = perm.tile([128, B], f32)
    aggT = perm.tile([128, cfg.NLP], f32)     # resident aggregation accumulator
    # W1b-projected x rows at two 64-node alignments (for the x[dst] expansion)
    PXa = perm.tile([128, cfg.NBLK, 128], bf16)
    PXb = perm.tile([128, cfg.NBLK, 128], bf16)

    ident_f = perm.tile([128, 128], f32)
    make_identity(nc, ident_f[:])
    ident_b = perm.tile([128, 128], bf16)
    nc.vector.tensor_copy(ident_b[:], ident_f[:])

    # ---------------- init DRAM state
    nc.sync.dma_start(t['eTd'][0][:], t['eT0'][:])
    nc.sync.dma_start(t['x_full'][:], t['x0b'][:])
    x_pair = t['x_full'].rearrange("(a two) h -> a (two h)", two=2)  # [N/2, 2H]

    def gru(xiT, hTb, wb, bb, pool, h_f32, out_tag, FD):
        """GRU tail: xiT bf16 [128,FD] (input through W2 fold), hTb bf16 [128,FD].
        If h_f32 given: blend in f32 in-place there and return None; else return
        a bf16 tile. wb: base index of Wih2 r,z,g then Whh r,z,g. bb: bias base."""
        pr = ps_g.tile([128, FD], f32, tag="pr")
        nc.tensor.matmul(pr[:], lhsT=w(wb + 0), rhs=xiT, start=True, stop=False)
        nc.tensor.matmul(pr[:], lhsT=w(wb + 3), rhs=hTb, start=False, stop=True)
        pz = ps_g.tile([128, FD], f32, tag="pz")
        nc.tensor.matmul(pz[:], lhsT=w(wb + 1), rhs=xiT, start=True, stop=False)
        nc.tensor.matmul(pz[:], lhsT=w(wb + 4), rhs=hTb, start=False, stop=True)
        pig = ps_g.tile([128, FD], f32, tag="pig")
        nc.tensor.matmul(pig[:], lhsT=w(wb + 2), rhs=xiT, start=True, stop=True)
        phg = ps_g.tile([128, FD], f32, tag="phg")
        nc.tensor.matmul(phg[:], lhsT=w(wb + 5), rhs=hTb, start=True, stop=True)

        r = pool.tile([128, FD], bf16, tag="r")
        nc.scalar.activation(r[:], pr[:], AF.Sigmoid, bias=bv(bb + 0))
        z = pool.tile([128, FD], bf16, tag="z")
        nc.scalar.activation(z[:], pz[:], AF.Sigmoid, bias=bv(bb + 1))
        hg = pool.tile([128, FD], bf16, tag="hg")
        nc.scalar.activation(hg[:], phg[:], AF.Identity, bias=bv(bb + 2))
        tm = pool.tile([128, FD], f32, tag="tm")
        nc.vector.tensor_tensor(tm[:], r[:], hg[:], op=ALU.mult)
        sp = pool.tile([128, FD], f32, tag="sp")
        nc.vector.tensor_tensor(sp[:], tm[:], pig[:], op=ALU.add)
        n = pool.tile([128, FD], bf16, tag="n")
        nc.scalar.activation(n[:], sp[:], AF.Tanh, bias=bv(bb + 3))

        hold = h_f32 if h_f32 is not None else hTb
        d = pool.tile([128, FD], f32, tag="d")
        nc.vector.tensor_tensor(d[:], hold, n[:], op=ALU.subtract)
        m = pool.tile([128, FD], f32, tag="m")
        nc.vector.tensor_tensor(m[:], z[:], d[:], op=ALU.mult)
        if h_f32 is not None:
            nc.vector.tensor_tensor(h_f32, n[:], m[:], op=ALU.add)
            return None
        hN = pool.tile([128, FD], bf16, tag=out_tag)
        nc.vector.tensor_tensor(hN[:], n[:], m[:], op=ALU.add)
        return hN

    # SWDGE queue assignment: Tile round-robins DMASW sems (8) over SWDGE
    # instructions in emission order; queue = ctr % num_queues keeps each sem
    # pinned to one queue (sem s -> queue s % 4).
    _swdge_ctr = [0]

    def self_qn(_):
        q = _swdge_ctr[0] % nc.num_swdge_queues
        _swdge_ctr[0] += 1
        return q

    aw_ps = None
    for s in range(cfg.STEPS):
        eT_r, eT_w = t['eTd'][s % 2], t['eTd'][(s + 1) % 2]
        nc.vector.memset(aggT[:], 0.0)

        # per-step u projections: uWd_row = u @ W1d.T ; uWnc_row = u @ Wn1c.T
        uprj = []
        for wi, tg in ((3, "uprj_e"), (12, "uprj_n")):
            p = ps_g.tile([B, 128], f32, tag="pr")
            nc.tensor.matmul(p[:], lhsT=uTb[:], rhs=w(wi), start=True, stop=True)
            srow = sb2.tile([B, 128], bf16, tag=tg)
            nc.vector.tensor_copy(srow[:], p[:])
            uprj.append(srow)
        uWd_row, uWnc_row = uprj

        # PXrow: per 128-node block, rows of x @ W1b.T (two 64-node alignments)
        for dstn, off in ((PXa, 0), (PXb, 64)):
            for blk in range(cfg.NBLK):
                base = off + blk * 128
                wid = min(128, cfg.NLP - base)
                if wid <= 0:
                    break
                px = ps_h1.tile([128, 128], f32, tag="h1")
                nc.tensor.matmul(px[:wid, :], lhsT=xTb[:, base:base + wid],
                                 rhs=w(1), start=True, stop=True)
                nc.vector.tensor_copy(dstn[:, blk, :][:wid, :], px[:wid, :])

        # ================= EDGE PHASE =================
        GB = min(2048, cfg.EPAD)          # gather batch (edges per dma_gather)
        CPB = GB // CH
        g_pair_b = None
        for k in range(cfg.NCHE):
            ce = slice(k * CH, (k + 1) * CH)

            if k % CPB == 0:
                cb = slice((k * CH) // 16, (k * CH + GB) // 16)
                ipr = sb.tile([128, GB // 16], i16, tag="ipr")
                nc.sync.dma_start(ipr[:], t['gpair'][:, cb])
                g_pair_b = sb.tile([128, 2, GB], bf16, tag="g_pair")
                nc.gpsimd.dma_gather(g_pair_b[:], x_pair, ipr[:], GB, GB, 2 * H,
                                     transpose=True, single_packet=False,
                                     queue_num=self_qn(0))

            kk = (k % CPB) * CH
            # parity merge in place: even slot := odd where src odd
            pm = sb.tile([128, CH], DT.uint8, tag="pm")
            nc.sync.dma_start(pm[:], t['pmaskT'][:, ce])
            nc.vector.copy_predicated(g_pair_b[:, 0, kk:kk + CH], pm[:],
                                      g_pair_b[:, 1, kk:kk + CH])
            g_src = g_pair_b[:, 0, kk:kk + CH]

            eT_c = sb.tile([128, CH], bf16, tag="eT_c")
            nc.sync.dma_start(eT_c[:], eT_r[:, ce])
            su_c = sb.tile([B, CH], bf16, tag="su_c")
            nc.sync.dma_start(su_c[:], t['S_u'][:, ce])
            d0 = sb.tile([128, CH], bf16, tag="d0")
            nc.sync.dma_start(d0[:], t['Dmat'][k, 0, :, :])
            d1 = sb.tile([128, CH], bf16, tag="d1")
            nc.sync.dma_start(d1[:], t['Dmat'][k, 1, :, :])

            w2 = cfg.w2start[k]
            if w2 % 128 == 0:
                pxh0 = PXa[:, w2 // 128, :]
                pxh1 = PXa[:, w2 // 128 + 1, :]
            else:
                pxh0 = PXb[:, (w2 - 64) // 128, :]
                pxh1 = PXb[:, (w2 - 64) // 128 + 1, :]

            h1 = ps_h1.tile([128, CH], f32, tag="h1")
            nc.tensor.matmul(h1[:], lhsT=w(0), rhs=g_src, start=True, stop=False)
            nc.tensor.matmul(h1[:], lhsT=pxh0, rhs=d0[:], start=False, stop=False)
            nc.tensor.matmul(h1[:], lhsT=pxh1, rhs=d1[:], start=False, stop=False)
            nc.tensor.matmul(h1[:], lhsT=w(2), rhs=eT_c[:], start=False, stop=False)
            nc.tensor.matmul(h1[:], lhsT=uWd_row[:], rhs=su_c[:], start=False, stop=True)

            rh1 = sb.tile([128, CH], bf16, tag="rh1")
            nc.scalar.activation(rh1[:], h1[:], AF.Relu, bias=bv(0))

            hN = gru(rh1[:], eT_c[:], 4, 1, sb, None, "hN", CH)
            nc.sync.dma_start(eT_w[:, ce], hN[:])

            # transpose to row-form, then aggregate via one-hot A matmuls
            tpp = ps_tp.tile([128, CH], bf16, tag="tp_b")
            for j in range(NSUB):
                nc.tensor.transpose(tpp[:, j * 128:(j + 1) * 128],
                                    hN[:, j * 128:(j + 1) * 128], ident_b[:])
            erow = sb.tile([128, CH], bf16, tag="erow")
            nc.vector.tensor_copy(erow[:], tpp[:])
            for j in range(NSUB):
                gs = k * NSUB + j
                wb = cfg.wstart[gs]
                first = (gs == 0) or (cfg.wstart[gs - 1] != wb)
                last = (gs == cfg.NSUBS - 1) or (cfg.wstart[gs + 1] != wb)
                atile = sb.tile([128, cfg.AW], bf16, tag="atile")
                nc.sync.dma_start(atile[:], t['Amat'][gs, :, :])
                if first:
                    aw_ps = ps_tp.tile([128, cfg.AW], f32, tag="aw")
                nc.tensor.matmul(aw_ps[:], lhsT=erow[:, j * 128:(j + 1) * 128],
                                 rhs=atile[:], start=first, stop=last)
                if last:
                    nc.vector.tensor_tensor(aggT[:, wb:wb + cfg.AW],
                                            aggT[:, wb:wb + cfg.AW],
                                            aw_ps[:], op=ALU.add)

        # ================= NODE PHASE =================
        for k in range(cfg.NCHN):
            cn = slice(k * CH, (k + 1) * CH)

            aggTb = sb.tile([128, CH], bf16, tag="aggTb")
            nc.vector.tensor_copy(aggTb[:], aggT[:, cn])

            snb_c = sb.tile([B, CH], bf16, tag="su_c")
            nc.sync.dma_start(snb_c[:], t['S_nb'][:, cn])

            h1 = ps_h1.tile([128, CH], f32, tag="h1")
            nc.tensor.matmul(h1[:], lhsT=w(10), rhs=xTb[:, cn], start=True, stop=False)
            nc.tensor.matmul(h1[:], lhsT=w(11), rhs=aggTb[:], start=False, stop=False)
            nc.tensor.matmul(h1[:], lhsT=uWnc_row[:], rhs=snb_c[:], start=False, stop=True)

            rh1 = sb.tile([128, CH], bf16, tag="rh1")
            nc.scalar.activation(rh1[:], h1[:], AF.Relu, bias=bv(5))

            gru(rh1[:], xTb[:, cn], 13, 6, sb, xT[:, cn], None, CH)
            nc.vector.tensor_copy(xTb[:, cn], xT[:, cn])

            # row-form x for AllGather input, local gather table, graph means
            bmat_c = sb.tile([128, NSUB, B], bf16, tag="bmat_c")
            for j in range(NSUB):
                nc.sync.dma_start(bmat_c[:, j, :],
                                  t['Bmat'][k * CH + j * 128: k * CH + (j + 1) * 128, :])
            bmm = ps_g.tile([128, B], f32, tag="pr")
            for j in range(NSUB):
                xtp = ps_tp.tile([128, 128], bf16, tag="tp_b")
                nc.tensor.transpose(xtp[:], xTb[:, k * CH + j * 128: k * CH + (j + 1) * 128],
                                    ident_b[:])
                xrow = sb.tile([128, 128], bf16, tag="xrow")
                nc.vector.tensor_copy(xrow[:], xtp[:])
                base = k * CH + j * 128
                nrows = max(0, min(128, cfg.NL - base))
                if nrows > 0 and s < cfg.STEPS - 1:
                    nc.sync.dma_start(t['x_shard'][base:base + nrows, :], xrow[:nrows, :])
                nc.tensor.matmul(bmm[:], lhsT=xrow[:], rhs=bmat_c[:, j, :],
                                 start=(j == 0), stop=(j == NSUB - 1))
            if k == 0:
                nc.vector.tensor_copy(bsum_acc[:], bmm[:])
            else:
                nc.vector.tensor_tensor(bsum_acc[:], bsum_acc[:], bmm[:], op=ALU.add)

        # ================= GLOBAL PHASE =================
        nc.sync.dma_start(t['gsum_in'][:], bsum_acc[:])
        nc.gpsimd.collective_compute(
            "AllReduce", ALU.add, replica_groups=t['rg'],
            ins=[t['gsum_in'][:]], outs=[t['gsum_out'][:]])
        nmF = sb2.tile([128, B], f32, tag="nmF")
        nc.sync.dma_start(nmF[:], t['gsum_out'][:])
        nmT = sb2.tile([128, B], bf16, tag="nmT")
        nc.vector.tensor_copy(nmT[:], nmF[:])

        h1g = ps_h1.tile([128, B], f32, tag="h1")
        nc.tensor.matmul(h1g[:], lhsT=w(19), rhs=uTb[:], start=True, stop=False)
        nc.tensor.matmul(h1g[:], lhsT=w(20), rhs=nmT[:], start=False, stop=True)
        rh1g = sb2.tile([128, B], bf16, tag="rh1g")
        nc.scalar.activation(rh1g[:], h1g[:], AF.Relu, bias=bv(10))

        gru(rh1g[:], uTb[:], 21, 11, sb2, uT[:], None, B)
        nc.vector.tensor_copy(uTb[:], uT[:])

        utp = ps_tp.tile([B, 128], f32, tag="aw")
        nc.tensor.transpose(utp[:], uT[:], ident_f[:])
        urow = sb2.tile([B, 128], f32, tag="urow")
        nc.vector.tensor_copy(urow[:], utp[:])
        nc.sync.dma_start(t['out'][:, s, :], urow[:])

        # ================= AllGather x (x_full doubles as the gather table) ==
        if s < cfg.STEPS - 1:
            nc.gpsimd.collective_compute(
                "AllGather", ALU.bypass, replica_groups=t['rg'],
                ins=[t['x_shard'][:]], outs=[t['x_full'][:]])


# ---------------------------------------------------------------- entry point

_CACHE = {}


def kernel(**inputs):
    x = np.asarray(inputs['x'])
    ei = np.asarray(inputs['edge_index'])
    u = np.asarray(inputs['u'])
    cfg = Cfg(N=x.shape[0], E=ei.shape[1], B=u.shape[0], H=x.shape[1], STEPS=3)
    in_maps = host_prepare(cfg, inputs)
    key = (cfg.N, cfg.E, cfg.B, cfg.H, cfg.STEPS, cfg.EPAD)
    if key not in _CACHE:
        _CACHE[key] = build_program(cfg)
    nc = _CACHE[key]
    res = run_bass_kernel_spmd(nc, in_maps, list(range(cfg.NCORES)))
    return np.asarray(res.results[0]["out"], np.float32)


# revision 47
# speedup vs baseline: 1.9696x; 1.0180x over previous
"""Trainium2 Bass kernel for MetaLayer-style GNN (edge/node/global GRU message passing).

Contract: kernel(**inputs) takes the FULL unsharded inputs (np arrays, keys as in
setup_inputs) and returns the FULL output [B, STEPS, H] float32.

Strategy (8 NeuronCores):
- Sort edges by dst, shard nodes into 8 equal contiguous ranges; each core owns all
  edges whose dst is in its range => node aggregation is core-local.
- Per step: edge MLP+GRU (edge-parallel, bf16 matmuls, T-form activations),
  local segment-sum via PE-transpose + dma_scatter_add (fp32, DMA CCE adds),
  node MLP+GRU on local nodes, AllGather of updated x (bf16) to rebuild the
  replicated gather tables, small AllReduce for per-graph node means, replicated
  global MLP+GRU on every core.
- x and u kept resident in fp32 SBUF; MLP second layer folded into GRU input
  weights: gi = relu_h1 @ (Wih@W2).T + (Wih@b2 + bih).
"""

from contextlib import ExitStack

import numpy as np
import ml_dtypes

import concourse.bass as bass
import concourse.bacc as bacc
import concourse.tile as tile
from concourse import mybir
from concourse.bass_utils import run_bass_kernel_spmd
from concourse.masks import make_identity

BF16 = ml_dtypes.bfloat16
AF = mybir.ActivationFunctionType
DT = mybir.dt
ALU = mybir.AluOpType

# ---------------------------------------------------------------- configuration

class Cfg:
    def __init__(self, N=50000, E=500000, B=64, H=128, STEPS=3, NCORES=8,
                 CH=512, SCB=4096):
        assert H == 128
        assert N % NCORES == 0
        self.N, self.E, self.B, self.H, self.STEPS, self.NCORES = N, E, B, H, STEPS, NCORES
        self.CH = CH                      # edge chunk (free dim of f32 PSUM <= 512)
        self.SCB = SCB                    # edges per dma_scatter_add call
        self.NL = N // NCORES             # local nodes
        self.NLP = ((self.NL + CH - 1) // CH) * CH
        self.NCHN = self.NLP // CH        # node chunks
        self.LO_REAL = min(N, 32767)      # x rows in lo table (int16 index limit)
        self.HI_REAL = N - self.LO_REAL
        self.LO_ROWS = self.LO_REAL + 1   # + zero row
        self.HI_ROWS = self.HI_REAL + 1   # + zero row
        self.TLOC_ROWS = self.NL + 1      # + zero row

    def finalize(self, max_shard_edges):
        assert self.SCB % self.CH == 0
        self.EPAD = ((max_shard_edges + self.SCB - 1) // self.SCB) * self.SCB
        self.NCHE = self.EPAD // self.CH  # edge chunks
        self.NSUBS = self.EPAD // 128     # 128-edge subs (one A tile each)
        self.AW = 256                     # aggregation window width (nodes)
        # data-independent window start per sub (aligned 128, clamped)
        self.wstart = []
        for sub in range(self.NSUBS):
            c = (sub + 0.5) * 128 * self.NL / self.EPAD
            w = 128 * int(c // 128) - 64
            w = max(0, min(w, self.NLP - self.AW))
            self.wstart.append(w)
        # chunk-level windows for the x[dst] expansion matmuls (64-aligned)
        self.w2start = []
        for k in range(self.NCHE):
            c = (k + 0.5) * self.CH * self.NL / self.EPAD
            w = 64 * int((c - 96) // 64)
            w = max(0, min(w, self.NLP - self.AW))
            self.w2start.append(w)
        self.NBLK = self.NLP // 128       # PXrow blocks
        return self


# ---------------------------------------------------------------- host helpers

def _wrap16(idx, call):
    """Pack indices into the wrapped-16, replicated-128 layout of dma_gather /
    dma_scatter_add: element [p, c*(call//16) + s] = idx[c*call + s*16 + p%16]."""
    total = idx.shape[0]
    assert total % call == 0 and call % 16 == 0
    ncalls = total // call
    w = idx.reshape(ncalls, call // 16, 16)                   # [c, s, lane]
    w = np.transpose(w, (2, 0, 1)).reshape(16, total // 16)   # [lane, c*s]
    w = np.tile(w, (8, 1))                                    # -> 128 partitions
    return np.ascontiguousarray(w.astype(np.int16))


def _onehot(cols_idx, nrows, scale=None, dtype=BF16):
    """[nrows, len(cols_idx)]: out[cols_idx[j], j] = scale_j; idx<0 -> zero col."""
    ncols = cols_idx.shape[0]
    out = np.zeros((nrows, ncols), dtype=np.float32)
    j = np.nonzero(cols_idx >= 0)[0]
    s = np.ones(j.shape[0], np.float32) if scale is None else scale[j]
    out[cols_idx[j], j] = s
    return out.astype(dtype)


def host_prepare(cfg, inputs):
    N, E, B, H = cfg.N, cfg.E, cfg.B, cfg.H
    x = np.asarray(inputs['x'], np.float32)
    edge_index = np.asarray(inputs['edge_index'])
    edge_attr = np.asarray(inputs['edge_attr'], np.float32)
    u = np.asarray(inputs['u'], np.float32)
    batch = np.asarray(inputs['batch']).astype(np.int64)
    src, dst = edge_index[0].astype(np.int64), edge_index[1].astype(np.int64)

    def g(name):
        return np.asarray(inputs[name], np.float32)

    W1, b1 = g('edge_w1'), g('edge_b1')
    W2, b2 = g('edge_w2'), g('edge_b2')
    eWih, eWhh = g('egru_wih'), g('egru_whh')
    eBih, eBhh = g('egru_bih'), g('egru_bhh')
    nW1, nb1 = g('node_w1'), g('node_b1')
    nW2, nb2 = g('node_w2'), g('node_b2')
    nWih, nWhh = g('ngru_wih'), g('ngru_whh')
    nBih, nBhh = g('ngru_bih'), g('ngru_bhh')
    gW1, gb1 = g('glob_w1'), g('glob_b1')
    gW2, gb2 = g('glob_w2'), g('glob_b2')
    gWih, gWhh = g('ggru_wih'), g('ggru_whh')
    gBih, gBhh = g('ggru_bih'), g('ggru_bhh')

    eWih2, eBih2 = eWih @ W2, eWih @ b2 + eBih
    nWih2, nBih2 = nWih @ nW2, nWih @ nb2 + nBih
    gWih2, gBih2 = gWih @ gW2, gWih @ gb2 + gBih

    def gate(Wm, i):
        return Wm[i * H:(i + 1) * H, :].T

    blocks = [
        W1[:, 0:H].T, W1[:, H:2 * H].T, W1[:, 2 * H:3 * H].T, W1[:, 3 * H:4 * H].T,
        gate(eWih2, 0), gate(eWih2, 1), gate(eWih2, 2),
        gate(eWhh, 0), gate(eWhh, 1), gate(eWhh, 2),
        nW1[:, 0:H].T, nW1[:, H:2 * H].T, nW1[:, 2 * H:3 * H].T,
        gate(nWih2, 0), gate(nWih2, 1), gate(nWih2, 2),
        gate(nWhh, 0), gate(nWhh, 1), gate(nWhh, 2),
        gW1[:, 0:H].T, gW1[:, H:2 * H].T,
        gate(gWih2, 0), gate(gWih2, 1), gate(gWih2, 2),
        gate(gWhh, 0), gate(gWhh, 1), gate(gWhh, 2),
    ]
    wpk = np.concatenate([bl.astype(np.float32) for bl in blocks], axis=1).astype(BF16)

    def gb_(v, i):
        return v[i * H:(i + 1) * H]

    bcols = [
        b1, gb_(eBih2, 0) + gb_(eBhh, 0), gb_(eBih2, 1) + gb_(eBhh, 1), gb_(eBhh, 2), gb_(eBih2, 2),
        nb1, gb_(nBih2, 0) + gb_(nBhh, 0), gb_(nBih2, 1) + gb_(nBhh, 1), gb_(nBhh, 2), gb_(nBih2, 2),
        gb1, gb_(gBih2, 0) + gb_(gBhh, 0), gb_(gBih2, 1) + gb_(gBhh, 1), gb_(gBhh, 2), gb_(gBih2, 2),
    ]
    bpk = np.stack(bcols, axis=1).astype(np.float32)

    order = np.argsort(dst, kind='stable')
    ssrc, sdst, sea = src[order], dst[order], edge_attr[order]
    shard_of = sdst // cfg.NL
    counts = np.bincount(shard_of, minlength=cfg.NCORES)
    cfg.finalize(int(counts.max()))

    gcnt = np.bincount(batch, minlength=B).astype(np.float32)
    ginv = 1.0 / np.maximum(gcnt, 1.0)
    ncnt = np.bincount(sdst, minlength=N).astype(np.float32)
    ninv = 1.0 / np.maximum(ncnt, 1.0)
    bsrc_all = batch[ssrc]

    xb = x.astype(BF16)
    in_maps = []
    bounds = np.searchsorted(sdst, np.arange(cfg.NCORES + 1) * cfg.NL)
    for c in range(cfg.NCORES):
        lo_, hi_ = int(bounds[c]), int(bounds[c + 1])
        ne = hi_ - lo_
        npad = cfg.EPAD - ne
        base = c * cfg.NL
        nl, nlp = cfg.NL, cfg.NLP

        # Interleave pads uniformly so slot->node quantile mapping matches the
        # program-uniform window schedule (all-at-end padding would drift).
        pad_slots = np.unique(np.round(np.linspace(0, cfg.EPAD - 1, npad)).astype(np.int64)) \
            if npad > 0 else np.empty(0, np.int64)
        # numerical safety: ensure exactly npad distinct slots
        while pad_slots.shape[0] < npad:
            extra = np.setdiff1d(np.arange(cfg.EPAD), pad_slots)[:npad - pad_slots.shape[0]]
            pad_slots = np.union1d(pad_slots, extra)
        is_pad = np.zeros(cfg.EPAD, bool)
        is_pad[pad_slots] = True
        slot_edge = np.full(cfg.EPAD, -1, np.int64)
        slot_edge[~is_pad] = np.arange(ne)

        def scatter_edges(vals, padval):
            out = np.full(cfg.EPAD, padval, vals.dtype)
            out[~is_pad] = vals
            return out

        csrc = ssrc[lo_:hi_]
        cdst_loc = sdst[lo_:hi_] - base
        cbsrc = bsrc_all[lo_:hi_]

        eslot = np.nonzero(~is_pad)[0]                     # slot of real edge i

        # src pair-gather: idx = src//2 into x viewed as [N/2, 2H]; merge parity
        gpair = scatter_edges(csrc // 2, np.int64(0))
        pmask = np.zeros(cfg.EPAD, np.float32)
        pmask[eslot] = (csrc % 2).astype(np.float32)
        pmaskT = np.ascontiguousarray(
            np.broadcast_to(pmask[None, :], (128, cfg.EPAD))).astype(np.uint8)

        # D tiles: per chunk, expansion one-hot [2, 128, CH] mapping window
        # nodes -> edge columns (x[dst] = PXrow_window contraction).
        w2 = np.asarray(cfg.w2start)                       # [NCHE]
        rel2 = cdst_loc - w2[eslot // cfg.CH]
        assert rel2.min() >= 0 and rel2.max() < cfg.AW, \
            f"dst window violated: {rel2.min()} {rel2.max()}"
        Dmat = np.zeros((cfg.NCHE, 2, 128, cfg.CH), np.float32)
        Dmat[eslot // cfg.CH, rel2 // 128, rel2 % 128, eslot % cfg.CH] = 1.0
        Dmat = Dmat.astype(BF16)

        # A tiles: per 128-edge sub, one-hot [128, AW] with 1/cnt folded,
        # targeting the sub's data-independent window.
        ws = np.asarray(cfg.wstart)                        # [NSUBS]
        rel = cdst_loc - ws[eslot // 128]
        assert rel.min() >= 0 and rel.max() < cfg.AW, \
            f"agg window violated: {rel.min()} {rel.max()}"
        Amat = np.zeros((cfg.NSUBS, 128, cfg.AW), np.float32)
        ninv_loc = ninv[base:base + nl]
        Amat[eslot // 128, eslot % 128, rel] = ninv_loc[cdst_loc]
        Amat = Amat.astype(BF16)

        batch_loc = batch[base:base + nl]
        bl_pad = np.concatenate([batch_loc, np.full(nlp - nl, -1, np.int64)])

        xT0 = np.zeros((128, nlp), np.float32)
        xT0[:, :nl] = x[base:base + nl].T
        eT0 = np.zeros((128, cfg.EPAD), BF16)
        eT0[:, eslot] = sea[lo_:hi_].T.astype(BF16)

        in_maps.append(dict(
            wpk=wpk, bpk=bpk,
            xT0=xT0,
            uT0=np.ascontiguousarray(u.T).astype(np.float32),
            eT0=eT0,
            x0b=xb,
            gpair=_wrap16(gpair, min(4096, cfg.EPAD)),
            pmaskT=pmaskT,
            Dmat=Dmat,
            Amat=Amat,
            S_u=_onehot(scatter_edges(cbsrc, np.int64(-1)), B),
            S_nb=_onehot(bl_pad, B),
            Bmat=np.ascontiguousarray(
                _onehot(bl_pad, B, scale=ginv[np.clip(bl_pad, 0, B - 1)]).T),
        ))
    return in_maps


# ---------------------------------------------------------------- device program

def build_program(cfg):
    nc = bacc.Bacc("TRN2", target_bir_lowering=False, debug=False,
                   num_devices=cfg.NCORES, num_swdge_queues=4)
    H, B, CH = cfg.H, cfg.B, cfg.CH
    NW = 27
    f32, bf16, i16 = DT.float32, DT.bfloat16, DT.int16

    def din(name, shape, dt):
        return nc.dram_tensor(name, shape, dt, kind="ExternalInput").ap()

    t = {}
    t['wpk'] = din("wpk", [128, NW * 128], bf16)
    t['bpk'] = din("bpk", [128, 15], f32)
    t['xT0'] = din("xT0", [128, cfg.NLP], f32)
    t['uT0'] = din("uT0", [128, B], f32)
    t['eT0'] = din("eT0", [128, cfg.EPAD], bf16)
    t['x0b'] = din("x0b", [cfg.N, H], bf16)
    t['gpair'] = din("gpair", [128, cfg.EPAD // 16], i16)
    t['pmaskT'] = din("pmaskT", [128, cfg.EPAD], DT.uint8)
    t['Dmat'] = din("Dmat", [cfg.NCHE, 2, 128, CH], bf16)
    t['Amat'] = din("Amat", [cfg.NSUBS, 128, cfg.AW], bf16)
    t['S_u'] = din("S_u", [B, cfg.EPAD], bf16)
    t['S_nb'] = din("S_nb", [B, cfg.NLP], bf16)
    t['Bmat'] = din("Bmat", [cfg.NLP, B], bf16)

    t['out'] = nc.dram_tensor("out", [B, cfg.STEPS, H], f32, kind="ExternalOutput").ap()

    t['eTd'] = [nc.dram_tensor(f"eTd{i}", [128, cfg.EPAD], bf16).ap() for i in range(2)]
    t['x_shard'] = nc.dram_tensor("x_shard", [cfg.NL, H], bf16).ap()
    t['x_full'] = nc.dram_tensor("x_full", [cfg.N, H], bf16, addr_space="Shared").ap()
    t['gsum_in'] = nc.dram_tensor("gsum_in", [128, B], f32).ap()
    t['gsum_out'] = nc.dram_tensor("gsum_out", [128, B], f32, addr_space="Shared").ap()
    t['rg'] = [list(range(cfg.NCORES))]

    with ExitStack() as ctx:
        tc = ctx.enter_context(tile.TileContext(nc))
        _emit(nc, tc, ctx, cfg, t)
    nc.compile()
    return nc


def _emit(nc, tc, ctx, cfg, t):
    H, B, CH = cfg.H, cfg.B, cfg.CH
    f32, bf16, i16 = DT.float32, DT.bfloat16, DT.int16
    NSUB = CH // 128

    perm = ctx.enter_context(tc.tile_pool(name="perm", bufs=1))
    sb = ctx.enter_context(tc.tile_pool(name="sb", bufs=3))
    sb2 = ctx.enter_context(tc.tile_pool(name="sb2", bufs=2))
    ps_h1 = ctx.enter_context(tc.tile_pool(name="ps_h1", bufs=2, space="PSUM"))
    ps_g = ctx.enter_context(tc.tile_pool(name="ps_g", bufs=1, space="PSUM"))
    ps_tp = ctx.enter_context(tc.tile_pool(name="ps_tp", bufs=1, space="PSUM"))

    # ---------------- persistent SBUF state
    W = perm.tile([128, 27 * 128], bf16)
    nc.sync.dma_start(W[:], t['wpk'][:])

    def w(i):
        return W[:, i * 128:(i + 1) * 128]

    bias = perm.tile([128, 15], f32)
    nc.sync.dma_start(bias[:], t['bpk'][:])

    def bv(i):
        return bias[:, i:i + 1]

    xT = perm.tile([128, cfg.NLP], f32)
    nc.sync.dma_start(xT[:], t['xT0'][:])
    xTb = perm.tile([128, cfg.NLP], bf16)
    nc.vector.tensor_copy(xTb[:], xT[:])

    uT = perm.tile([128, B], f32)
    nc.sync.dma_start(uT[:], t['uT0'][:])
    uTb = perm.tile([128, B], bf16)
    nc.vector.tensor_copy(uTb[:], uT[:])

    bsum_acc = perm.tile([128, B], f32)
    aggT = perm.tile([128, cfg.NLP], f32)     # resident aggregation accumulator
    # W1b-projected x rows at two 64-node alignments (for the x[dst] expansion)
    PXa = perm.tile([128, cfg.NBLK, 128], bf16)
    PXb = perm.tile([128, cfg.NBLK, 128], bf16)

    ident_f = perm.tile([128, 128], f32)
    make_identity(nc, ident_f[:])
    ident_b = perm.tile([128, 128], bf16)
    nc.vector.tensor_copy(ident_b[:], ident_f[:])

    # ---------------- init DRAM state
    nc.sync.dma_start(t['eTd'][0][:], t['eT0'][:])
    nc.sync.dma_start(t['x_full'][:], t['x0b'][:])
    x_pair = t['x_full'].rearrange("(a two) h -> a (two h)", two=2)  # [N/2, 2H]

    def gru(xiT, hTb, wb, bb, pool, h_f32, out_tag, FD):
        """GRU tail: xiT bf16 [128,FD] (input through W2 fold), hTb bf16 [128,FD].
        If h_f32 given: blend in f32 in-place there and return None; else return
        a bf16 tile. wb: base index of Wih2 r,z,g then Whh r,z,g. bb: bias base."""
        pr = ps_g.tile([128, FD], f32, tag="pr")
        nc.tensor.matmul(pr[:], lhsT=w(wb + 0), rhs=xiT, start=True, stop=False)
        nc.tensor.matmul(pr[:], lhsT=w(wb + 3), rhs=hTb, start=False, stop=True)
        pz = ps_g.tile([128, FD], f32, tag="pz")
        nc.tensor.matmul(pz[:], lhsT=w(wb + 1), rhs=xiT, start=True, stop=False)
        nc.tensor.matmul(pz[:], lhsT=w(wb + 4), rhs=hTb, start=False, stop=True)
        pig = ps_g.tile([128, FD], f32, tag="pig")
        nc.tensor.matmul(pig[:], lhsT=w(wb + 2), rhs=xiT, start=True, stop=True)
        phg = ps_g.tile([128, FD], f32, tag="phg")
        nc.tensor.matmul(phg[:], lhsT=w(wb + 5), rhs=hTb, start=True, stop=True)

        r = pool.tile([128, FD], bf16, tag="r")
        nc.scalar.activation(r[:], pr[:], AF.Sigmoid, bias=bv(bb + 0))
        z = pool.tile([128, FD], bf16, tag="z")
        nc.scalar.activation(z[:], pz[:], AF.Sigmoid, bias=bv(bb + 1))
        hg = pool.tile([128, FD], bf16, tag="hg")
        nc.scalar.activation(hg[:], phg[:], AF.Identity, bias=bv(bb + 2))
        tm = pool.tile([128, FD], f32, tag="tm")
        nc.vector.tensor_tensor(tm[:], r[:], hg[:], op=ALU.mult)
        sp = pool.tile([128, FD], f32, tag="sp")
        nc.vector.tensor_tensor(sp[:], tm[:], pig[:], op=ALU.add)
        n = pool.tile([128, FD], bf16, tag="n")
        nc.scalar.activation(n[:], sp[:], AF.Tanh, bias=bv(bb + 3))

        hold = h_f32 if h_f32 is not None else hTb
        d = pool.tile([128, FD], f32, tag="d")
        nc.vector.tensor_tensor(d[:], hold, n[:], op=ALU.subtract)
        m = pool.tile([128, FD], f32, tag="m")
        nc.vector.tensor_tensor(m[:], z[:], d[:], op=ALU.mult)
        if h_f32 is not None:
            nc.vector.tensor_tensor(h_f32, n[:], m[:], op=ALU.add)
            return None
        hN = pool.tile([128, FD], bf16, tag=out_tag)
        nc.vector.tensor_tensor(hN[:], n[:], m[:], op=ALU.add)
        return hN

    # SWDGE queue assignment: Tile round-robins DMASW sems (8) over SWDGE
    # instructions in emission order; queue = ctr % num_queues keeps each sem
    # pinned to one queue (sem s -> queue s % 4).
    _swdge_ctr = [0]

    def self_qn(_):
        q = _swdge_ctr[0] % nc.num_swdge_queues
        _swdge_ctr[0] += 1
        return q

    aw_ps = None
    for s in range(cfg.STEPS):
        eT_r, eT_w = t['eTd'][s % 2], t['eTd'][(s + 1) % 2]
        nc.vector.memset(aggT[:], 0.0)

        # per-step u projections: uWd_row = u @ W1d.T ; uWnc_row = u @ Wn1c.T
        uprj = []
        for wi, tg in ((3, "uprj_e"), (12, "uprj_n")):
            p = ps_g.tile([B, 128], f32, tag="pr")
            nc.tensor.matmul(p[:], lhsT=uTb[:], rhs=w(wi), start=True, stop=True)
            srow = sb2.tile([B, 128], bf16, tag=tg)
            nc.vector.tensor_copy(srow[:], p[:])
            uprj.append(srow)
        uWd_row, uWnc_row = uprj

        # PXrow: per 128-node block, rows of x @ W1b.T (two 64-node alignments)
        for dstn, off in ((PXa, 0), (PXb, 64)):
            for blk in range(cfg.NBLK):
                base = off + blk * 128
                wid = min(128, cfg.NLP - base)
                if wid <= 0:
                    break
                px = ps_h1.tile([128, 128], f32, tag="h1")
                nc.tensor.matmul(px[:wid, :], lhsT=xTb[:, base:base + wid],
                                 rhs=w(1), start=True, stop=True)
                nc.vector.tensor_copy(dstn[:, blk, :][:wid, :], px[:wid, :])

        # ================= EDGE PHASE =================
        GB = min(4096, cfg.EPAD)          # gather batch (edges per dma_gather)
        CPB = GB // CH
        g_pair_b = None
        for k in range(cfg.NCHE):
            ce = slice(k * CH, (k + 1) * CH)

            if k % CPB == 0:
                cb = slice((k * CH) // 16, (k * CH + GB) // 16)
                ipr = sb.tile([128, GB // 16], i16, tag="ipr")
                nc.sync.dma_start(ipr[:], t['gpair'][:, cb])
                g_pair_b = sb.tile([128, 2, GB], bf16, tag="g_pair", bufs=2)
                nc.gpsimd.dma_gather(g_pair_b[:], x_pair, ipr[:], GB, GB, 2 * H,
                                     transpose=True, single_packet=False,
                                     queue_num=self_qn(0))

            kk = (k % CPB) * CH
            # parity merge in place: even slot := odd where src odd
            pm = sb.tile([128, CH], DT.uint8, tag="pm")
            nc.sync.dma_start(pm[:], t['pmaskT'][:, ce])
            nc.vector.copy_predicated(g_pair_b[:, 0, kk:kk + CH], pm[:],
                                      g_pair_b[:, 1, kk:kk + CH])
            g_src = g_pair_b[:, 0, kk:kk + CH]

            eT_c = sb.tile([128, CH], bf16, tag="eT_c")
            nc.sync.dma_start(eT_c[:], eT_r[:, ce])
            su_c = sb.tile([B, CH], bf16, tag="su_c")
            nc.sync.dma_start(su_c[:], t['S_u'][:, ce])
            d0 = sb.tile([128, CH], bf16, tag="d0")
            nc.sync.dma_start(d0[:], t['Dmat'][k, 0, :, :])
            d1 = sb.tile([128, CH], bf16, tag="d1")
            nc.sync.dma_start(d1[:], t['Dmat'][k, 1, :, :])

            w2 = cfg.w2start[k]
            if w2 % 128 == 0:
                pxh0 = PXa[:, w2 // 128, :]
                pxh1 = PXa[:, w2 // 128 + 1, :]
            else:
                pxh0 = PXb[:, (w2 - 64) // 128, :]
                pxh1 = PXb[:, (w2 - 64) // 128 + 1, :]

            h1 = ps_h1.tile([128, CH], f32, tag="h1")
            nc.tensor.matmul(h1[:], lhsT=w(0), rhs=g_src, start=True, stop=False)
            nc.tensor.matmul(h1[:], lhsT=pxh0, rhs=d0[:], start=False, stop=False)
            nc.tensor.matmul(h1[:], lhsT=pxh1, rhs=d1[:], start=False, stop=False)
            nc.tensor.matmul(h1[:], lhsT=w(2), rhs=eT_c[:], start=False, stop=False)
            nc.tensor.matmul(h1[:], lhsT=uWd_row[:], rhs=su_c[:], start=False, stop=True)

            rh1 = sb.tile([128, CH], bf16, tag="rh1")
            nc.scalar.activation(rh1[:], h1[:], AF.Relu, bias=bv(0))

            hN = gru(rh1[:], eT_c[:], 4, 1, sb, None, "hN", CH)
            nc.sync.dma_start(eT_w[:, ce], hN[:])

            # transpose to row-form, then aggregate via one-hot A matmuls
            tpp = ps_tp.tile([128, CH], bf16, tag="tp_b")
            for j in range(NSUB):
                nc.tensor.transpose(tpp[:, j * 128:(j + 1) * 128],
                                    hN[:, j * 128:(j + 1) * 128], ident_b[:])
            erow = sb.tile([128, CH], bf16, tag="erow")
            nc.vector.tensor_copy(erow[:], tpp[:])
            for j in range(NSUB):
                gs = k * NSUB + j
                wb = cfg.wstart[gs]
                first = (gs == 0) or (cfg.wstart[gs - 1] != wb)
                last = (gs == cfg.NSUBS - 1) or (cfg.wstart[gs + 1] != wb)
                atile = sb.tile([128, cfg.AW], bf16, tag="atile")
                nc.sync.dma_start(atile[:], t['Amat'][gs, :, :])
                if first:
                    aw_ps = ps_tp.tile([128, cfg.AW], f32, tag="aw")
                nc.tensor.matmul(aw_ps[:], lhsT=erow[:, j * 128:(j + 1) * 128],
                                 rhs=atile[:], start=first, stop=last)
                if last:
                    nc.vector.tensor_tensor(aggT[:, wb:wb + cfg.AW],
                                            aggT[:, wb:wb + cfg.AW],
                                            aw_ps[:], op=ALU.add)

        # ================= NODE PHASE =================
        for k in range(cfg.NCHN):
            cn = slice(k * CH, (k + 1) * CH)

            aggTb = sb.tile([128, CH], bf16, tag="aggTb")
            nc.vector.tensor_copy(aggTb[:], aggT[:, cn])

            snb_c = sb.tile([B, CH], bf16, tag="su_c")
            nc.sync.dma_start(snb_c[:], t['S_nb'][:, cn])

            h1 = ps_h1.tile([128, CH], f32, tag="h1")
            nc.tensor.matmul(h1[:], lhsT=w(10), rhs=xTb[:, cn], start=True, stop=False)
            nc.tensor.matmul(h1[:], lhsT=w(11), rhs=aggTb[:], start=False, stop=False)
            nc.tensor.matmul(h1[:], lhsT=uWnc_row[:], rhs=snb_c[:], start=False, stop=True)

            rh1 = sb.tile([128, CH], bf16, tag="rh1")
            nc.scalar.activation(rh1[:], h1[:], AF.Relu, bias=bv(5))

            gru(rh1[:], xTb[:, cn], 13, 6, sb, xT[:, cn], None, CH)
            nc.vector.tensor_copy(xTb[:, cn], xT[:, cn])

            # row-form x for AllGather input, local gather table, graph means
            bmat_c = sb.tile([128, NSUB, B], bf16, tag="bmat_c")
            for j in range(NSUB):
                nc.sync.dma_start(bmat_c[:, j, :],
                                  t['Bmat'][k * CH + j * 128: k * CH + (j + 1) * 128, :])
            bmm = ps_g.tile([128, B], f32, tag="pr")
            for j in range(NSUB):
                xtp = ps_tp.tile([128, 128], bf16, tag="tp_b")
                nc.tensor.transpose(xtp[:], xTb[:, k * CH + j * 128: k * CH + (j + 1) * 128],
                                    ident_b[:])
                xrow = sb.tile([128, 128], bf16, tag="xrow")
                nc.vector.tensor_copy(xrow[:], xtp[:])
                base = k * CH + j * 128
                nrows = max(0, min(128, cfg.NL - base))
                if nrows > 0 and s < cfg.STEPS - 1:
                    nc.sync.dma_start(t['x_shard'][base:base + nrows, :], xrow[:nrows, :])
                nc.tensor.matmul(bmm[:], lhsT=xrow[:], rhs=bmat_c[:, j, :],
                                 start=(j == 0), stop=(j == NSUB - 1))
            if k == 0:
                nc.vector.tensor_copy(bsum_acc[:], bmm[:])
            else:
                nc.vector.tensor_tensor(bsum_acc[:], bsum_acc[:], bmm[:], op=ALU.add)

        # ================= GLOBAL PHASE =================
        nc.sync.dma_start(t['gsum_in'][:], bsum_acc[:])
        nc.gpsimd.collective_compute(
            "AllReduce", ALU.add, replica_groups=t['rg'],
            ins=[t['gsum_in'][:]], outs=[t['gsum_out'][:]])
        nmF = sb2.tile([128, B], f32, tag="nmF")
        nc.sync.dma_start(nmF[:], t['gsum_out'][:])
        nmT = sb2.tile([128, B], bf16, tag="nmT")
        nc.vector.tensor_copy(nmT[:], nmF[:])

        h1g = ps_h1.tile([128, B], f32, tag="h1")
        nc.tensor.matmul(h1g[:], lhsT=w(19), rhs=uTb[:], start=True, stop=False)
        nc.tensor.matmul(h1g[:], lhsT=w(20), rhs=nmT[:], start=False, stop=True)
        rh1g = sb2.tile([128, B], bf16, tag="rh1g")
        nc.scalar.activation(rh1g[:], h1g[:], AF.Relu, bias=bv(10))

        gru(rh1g[:], uTb[:], 21, 11, sb2, uT[:], None, B)
        nc.vector.tensor_copy(uTb[:], uT[:])

        utp = ps_tp.tile([B, 128], f32, tag="aw")
        nc.tensor.transpose(utp[:], uT[:], ident_f[:])
        urow = sb2.tile([B, 128], f32, tag="urow")
        nc.vector.tensor_copy(urow[:], utp[:])
        nc.sync.dma_start(t['out'][:, s, :], urow[:])

        # ================= AllGather x (x_full doubles as the gather table) ==
        if s < cfg.STEPS - 1:
            nc.gpsimd.collective_compute(
                "AllGather", ALU.bypass, replica_groups=t['rg'],
                ins=[t['x_shard'][:]], outs=[t['x_full'][:]])


# ---------------------------------------------------------------- entry point

_CACHE = {}


def kernel(**inputs):
    x = np.asarray(inputs['x'])
    ei = np.asarray(inputs['edge_index'])
    u = np.asarray(inputs['u'])
    cfg = Cfg(N=x.shape[0], E=ei.shape[1], B=u.shape[0], H=x.shape[1], STEPS=3)
    in_maps = host_prepare(cfg, inputs)
    key = (cfg.N, cfg.E, cfg.B, cfg.H, cfg.STEPS, cfg.EPAD)
    if key not in _CACHE:
        _CACHE[key] = build_program(cfg)
    nc = _CACHE[key]
    res = run_bass_kernel_spmd(nc, in_maps, list(range(cfg.NCORES)))
    return np.asarray(res.results[0]["out"], np.float32)
